# revision 1
# baseline (speedup 1.0000x reference)
"""CompressiveMemory (Infini-attention style) Trainium2 Bass kernel.

Sharding: 8 cores = batch(2) x head-quad(4). Core c handles batch b=c//4 and
heads [4*(c%4), 4*(c%4)+4). The reference's `att.reshape(B, SEG, H*DV)` is a
torch-style view of the contiguous (B,H,SEG,DV) array, so segment-output row
r = h*32 + s//16 depends on ONE head only: each core produces rows
[128*(c%4), 128*(c%4)+128) of every 512-row segment block, and the host
gather is a pure concat (no cross-core reduction).

Per-core per-segment compute (all layouts chosen so no activation transposes
are needed):
  qT/kT = W^T @ xT-slice        [chan, tok]   (fp32r matmuls)
  v     = xT-slice^T @ Wv       [tok, chan]
  per head: scoresT = kT^T qT; e = exp((scoresT+mask)/sqrt(dk));
            den = ones^T e; U = v^T e; sigma_q/k = elu()+1;
            R = mem^T sigma_q; zden = z^T sigma_q;
            attT = U/den + beta*(R/zden - U/den)
            retz = sigma_kT^T [mem|z]; ndelta = ret/kvden - v;
            mem -= sigma_k_nat^T ndelta; z += rowsum(sigma_kT)
  out rows = scrambled-view(attT) @ Wo   (fp16 matmuls, full Wo resident)
"""
import numpy as np

import concourse.bass as bass
import concourse.mybir as mybir
import concourse.tile as tile
from concourse import bacc
from concourse.masks import make_identity

B, S, D = 2, 4096, 2048
H, DK, DV = 16, 128, 128
SEG = 512
NSEG = S // SEG
NCORE = 8
HPC = 4                      # heads per core
CH = HPC * DK                # 512 per-core q/k/v channels
SCALE = float(DK) ** -0.5
MASKVAL = -4.0e5             # pre-scale additive mask; exp((s+M)*SCALE) -> 0

f32 = mybir.dt.float32
f32r = mybir.dt.float32r
f16 = mybir.dt.float16
ALU = mybir.AluOpType
ACTF = mybir.ActivationFunctionType
AXIS = mybir.AxisListType

_MODULE_CACHE = {}


def _build_module():
    nc = bacc.Bacc("TRN2", target_bir_lowering=False, debug=False,
                   num_devices=NCORE)
    xT_d = nc.dram_tensor("xT", [D, S], f32r, kind="ExternalInput")
    wq_d = nc.dram_tensor("wq", [D, CH], f32r, kind="ExternalInput")
    wk_d = nc.dram_tensor("wk", [D, CH], f32r, kind="ExternalInput")
    wv_d = nc.dram_tensor("wv", [D, CH], f32r, kind="ExternalInput")
    wo_d = nc.dram_tensor("wo", [D, D], f16, kind="ExternalInput")
    mask_d = nc.dram_tensor("mask", [SEG, SEG], f32, kind="ExternalInput")
    beta_d = nc.dram_tensor("beta", [DV, HPC], f32, kind="ExternalInput")
    out_d = nc.dram_tensor("out", [NSEG, 128, D], f32, kind="ExternalOutput")

    with tile.TileContext(nc) as tc:
        _body(nc, tc, xT_d, wq_d, wk_d, wv_d, wo_d, mask_d, beta_d, out_d)
    nc.compile()
    return nc


def _body(nc, tc, xT_d, wq_d, wk_d, wv_d, wo_d, mask_d, beta_d, out_d):
    with (
        tc.tile_pool(name="statics", bufs=1) as st,
        tc.tile_pool(name="xt", bufs=16) as xt_pool,
        tc.tile_pool(name="wt", bufs=6) as wt_pool,
        tc.tile_pool(name="qkv", bufs=4) as qkv_pool,
        tc.tile_pool(name="sig", bufs=2) as sig_pool,
        tc.tile_pool(name="tmp", bufs=6) as tmp_pool,
        tc.tile_pool(name="exps", bufs=4) as exps_pool,
        tc.tile_pool(name="attp", bufs=2) as att_pool,
        tc.tile_pool(name="ndp", bufs=4) as nd_pool,
        tc.tile_pool(name="rvec", bufs=3) as rv_pool,
        tc.tile_pool(name="tiny", bufs=6) as tiny_pool,
        tc.tile_pool(name="outs", bufs=4) as out_pool,
        tc.tile_pool(name="mm", bufs=5, space=bass.MemorySpace.PSUM) as pp,
        tc.tile_pool(name="aux", bufs=3, space=bass.MemorySpace.PSUM) as pa,
    ):
        # ---- statics ----
        wo_sb = st.tile([128, 16 * D], f16, tag="wo")
        for j in range(16):
            nc.sync.dma_start(out=wo_sb[:, j * D:(j + 1) * D],
                              in_=wo_d[j * 128:(j + 1) * 128, :])
        mask_sb = st.tile([128, 4 * SEG], f32, tag="mask")
        for c4 in range(4):
            nc.sync.dma_start(out=mask_sb[:, c4 * SEG:(c4 + 1) * SEG],
                              in_=mask_d[c4 * 128:(c4 + 1) * 128, :])
        beta_sb = st.tile([DV, HPC], f32, tag="beta")
        nc.sync.dma_start(out=beta_sb[:], in_=beta_d[:])
        ident = st.tile([128, 128], f32, tag="ident")
        make_identity(nc, ident[:])
        # f32r cannot be memset directly: stage in f32, copy (copy rounds).
        ones32f = st.tile([128, 32], f32, tag="ones32f")
        nc.vector.memset(ones32f[:], 1.0)
        ones32 = st.tile([128, 32], f32r, tag="ones32")
        nc.vector.tensor_copy(ones32[:], ones32f[:])
        invf = st.tile([32, 128], f32, tag="invf")
        nc.vector.memset(invf[:], 1.0 / 32.0)
        inv32 = st.tile([32, 128], f32r, tag="inv32")
        nc.vector.tensor_copy(inv32[:], invf[:])
        # per-head memory state [dk, mem(128) | z(1) | zero-pad(127)]
        mzf = st.tile([128, 256], f32, tag="mzf")
        nc.vector.memset(mzf[:], 0.0)
        nc.vector.memset(mzf[:, 128:129], 1.0 / DK)
        mem_sb = []
        for h in range(HPC):
            m = st.tile([128, 256], f32r, tag=f"mem{h}")
            nc.vector.tensor_copy(m[:], mzf[:])
            mem_sb.append(m)

        # ---- main loop ----
        for seg in range(NSEG):
            # xT slice tiles [d-tile 128, SEG]
            xt = []
            for i in range(16):
                t = xt_pool.tile([128, SEG], f32r, tag="xt")
                nc.sync.dma_start(
                    out=t[:], in_=xT_d[i * 128:(i + 1) * 128,
                                       seg * SEG:(seg + 1) * SEG])
                xt.append(t)

            def proj_T(w_d, dtag):
                """qT/kT: [chan, tok] in 4 chunks of [128, SEG]."""
                dests = []
                ps = [pp.tile([128, SEG], f32, tag="mm", name=f"ps_{dtag}{c}")
                      for c in range(4)]
                for i in range(16):
                    w = wt_pool.tile([128, CH], f32r, tag="wt")
                    nc.sync.dma_start(out=w[:],
                                      in_=w_d[i * 128:(i + 1) * 128, :])
                    for c in range(4):
                        nc.tensor.matmul(ps[c][:],
                                         w[:, c * 128:(c + 1) * 128],
                                         xt[i][:],
                                         start=(i == 0), stop=(i == 15))
                for c in range(4):
                    dst = qkv_pool.tile([128, SEG], f32r, tag=dtag)
                    nc.vector.tensor_copy(dst[:], ps[c][:])
                    dests.append(dst)
                return dests

            def proj_N(w_d, dtag):
                """v: [tok, chan] in 4 token-chunks of [128, CH]."""
                dests = []
                ps = [pp.tile([128, CH], f32, tag="mm", name=f"ps_{dtag}{c}")
                      for c in range(4)]
                for i in range(16):
                    w = wt_pool.tile([128, CH], f32r, tag="wt")
                    nc.sync.dma_start(out=w[:],
                                      in_=w_d[i * 128:(i + 1) * 128, :])
                    for c in range(4):
                        nc.tensor.matmul(ps[c][:],
                                         xt[i][:, c * 128:(c + 1) * 128],
                                         w[:],
                                         start=(i == 0), stop=(i == 15))
                for c in range(4):
                    dst = qkv_pool.tile([128, CH], f32r, tag=dtag)
                    nc.scalar.copy(dst[:], ps[c][:])
                    dests.append(dst)
                return dests

            qT = proj_T(wq_d, "qT")
            kT = proj_T(wk_d, "kT")
            v = proj_N(wv_d, "v")

            attT = att_pool.tile([128, HPC * SEG], f16, tag="attT")

            for h in range(HPC):
                memh = mem_sb[h]

                def elu1(src, dtag, accum=None):
                    """sigma = elu(src)+1 = exp(min(src,0)) + relu(src)."""
                    mn = tmp_pool.tile([128, SEG], f32, tag="tmp")
                    nc.vector.tensor_scalar_min(mn[:], src[:], 0.0)
                    e = tmp_pool.tile([128, SEG], f32, tag="tmp")
                    nc.scalar.activation(e[:], mn[:], ACTF.Exp)
                    r = tmp_pool.tile([128, SEG], f32, tag="tmp")
                    nc.scalar.activation(r[:], src[:], ACTF.Relu)
                    out = sig_pool.tile([128, SEG], f32r, tag=dtag)
                    nc.vector.tensor_add(out[:], e[:], r[:])
                    return out

                sgq = elu1(qT[h], "sgq")
                sgk = elu1(kT[h], "sgk")
                # z increment = rowsum of sigma_kT over tokens
                zsum = tiny_pool.tile([128, 1], f32, tag="zsum")
                nc.vector.reduce_sum(zsum[:], sgk[:], axis=AXIS.X)
                # sigma_k natural layout via PE transpose
                signat = sig_pool.tile([128, SEG], f32r, tag="signat")
                for c4 in range(4):
                    pt = pa.tile([128, 128], f32, tag="aux")
                    nc.tensor.transpose(pt[:],
                                        sgk[:, c4 * 128:(c4 + 1) * 128].bitcast(f32),
                                        ident[:])
                    nc.vector.tensor_copy(
                        signat[:, c4 * 128:(c4 + 1) * 128], pt[:])

                # scoresT chunks -> exp((S+mask)*SCALE)
                es = []
                for c4 in range(4):
                    psc = pp.tile([128, SEG], f32, tag="mm")
                    nc.tensor.matmul(psc[:],
                                     kT[h][:, c4 * 128:(c4 + 1) * 128],
                                     qT[h][:])
                    nc.vector.tensor_tensor(
                        psc[:], psc[:],
                        mask_sb[:, c4 * SEG:(c4 + 1) * SEG], op=ALU.add)
                    e = exps_pool.tile([128, SEG], f32r, tag="exps")
                    nc.scalar.activation(e[:], psc[:], ACTF.Exp, scale=SCALE)
                    es.append(e)

                pden = pa.tile([32, SEG], f32, tag="aux")
                for c4 in range(4):
                    nc.tensor.matmul(pden[:], ones32[:], es[c4][:],
                                     start=(c4 == 0), stop=(c4 == 3))
                pU = pp.tile([128, SEG], f32, tag="mm")
                for c4 in range(4):
                    nc.tensor.matmul(pU[:],
                                     v[c4][:, h * 128:(h + 1) * 128],
                                     es[c4][:],
                                     start=(c4 == 0), stop=(c4 == 3))
                pR = pp.tile([128, SEG], f32, tag="mm")
                nc.tensor.matmul(pR[:], memh[:, 0:128], sgq[:])
                # zden rows: replicate z into 32 cols, then M=32 matmul
                zrep = tiny_pool.tile([128, 32], f32r, tag="zrep")
                nc.vector.tensor_scalar_mul(zrep[:], ones32f[:],
                                            memh[:, 128:129].bitcast(f32))
                pzd = pa.tile([32, SEG], f32, tag="aux")
                nc.tensor.matmul(pzd[:], zrep[:], sgq[:])

                rden = rv_pool.tile([32, SEG], f32r, tag="rvec")
                rzden = rv_pool.tile([32, SEG], f32r, tag="rvec")
                with nc.allow_low_precision(reason="fp32r for PE broadcast"):
                    nc.vector.reciprocal(rden[:], pden[:])
                    nc.vector.reciprocal(rzden[:], pzd[:])
                pbd = pp.tile([128, SEG], f32, tag="mm")
                nc.tensor.matmul(pbd[:], inv32[:], rden[:])
                pbz = pp.tile([128, SEG], f32, tag="mm")
                nc.tensor.matmul(pbz[:], inv32[:], rzden[:])

                # DVE cannot read two PSUM operands in one op: stage the
                # broadcasts through SBUF on the scalar engine first.
                bd = tmp_pool.tile([128, SEG], f32, tag="tmp")
                nc.scalar.copy(bd[:], pbd[:])
                bz = tmp_pool.tile([128, SEG], f32, tag="tmp")
                nc.scalar.copy(bz[:], pbz[:])
                t1 = tmp_pool.tile([128, SEG], f32, tag="tmp")
                nc.vector.tensor_tensor(t1[:], pU[:], bd[:], op=ALU.mult)
                t2 = tmp_pool.tile([128, SEG], f32, tag="tmp")
                nc.vector.tensor_tensor(t2[:], pR[:], bz[:], op=ALU.mult)
                nc.vector.tensor_sub(t2[:], t2[:], t1[:])
                nc.vector.scalar_tensor_tensor(
                    attT[:, h * SEG:(h + 1) * SEG],
                    t2[:], beta_sb[:, h:h + 1], t1[:],
                    op0=ALU.mult, op1=ALU.add)

                # ---- memory update (delta rule) ----
                pmu = pa.tile([128, 128], f32, tag="aux")
                for c4 in range(4):
                    prz = pa.tile([128, 256], f32, tag="aux")
                    nc.tensor.matmul(prz[:],
                                     sgk[:, c4 * 128:(c4 + 1) * 128],
                                     memh[:])
                    rk = tiny_pool.tile([128, 1], f32, tag="rk")
                    nc.vector.reciprocal(rk[:], prz[:, 128:129])
                    nd = nd_pool.tile([128, 128], f32r, tag="nd")
                    nc.vector.scalar_tensor_tensor(
                        nd[:], prz[:, 0:128], rk[:],
                        v[c4][:, h * 128:(h + 1) * 128],
                        op0=ALU.mult, op1=ALU.subtract)
                    nc.tensor.matmul(pmu[:],
                                     signat[:, c4 * 128:(c4 + 1) * 128],
                                     nd[:],
                                     start=(c4 == 0), stop=(c4 == 3))
                nc.vector.tensor_sub(memh[:, 0:128], memh[:, 0:128], pmu[:])
                nc.vector.tensor_tensor(memh[:, 128:129], memh[:, 128:129],
                                        zsum[:], op=ALU.add)

            # ---- output projection (torch-view scramble baked into the AP) ----
            # row r = h*32+g <- attT column h*512 + 16*g + j, contracted over
            # (j, v) against Wo rows j*128+v.
            attv = attT[:].rearrange("p (h g j) -> p h g j", h=HPC, g=32, j=16)
            for o in range(4):
                po = pp.tile([128, 512], f32, tag="mm")
                for j in range(16):
                    nc.tensor.matmul(
                        po[:], attv[:, :, :, j],
                        wo_sb[:, j * D + o * 512: j * D + o * 512 + 512],
                        start=(j == 0), stop=(j == 15))
                osb = out_pool.tile([128, 512], f32, tag="outs")
                if o % 2 == 0:
                    nc.scalar.copy(osb[:], po[:])
                else:
                    nc.vector.tensor_copy(osb[:], po[:])
                nc.sync.dma_start(out=out_d[seg, :, o * 512:(o + 1) * 512],
                                  in_=osb[:])


def get_module():
    if "nc" not in _MODULE_CACHE:
        _MODULE_CACHE["nc"] = _build_module()
    return _MODULE_CACHE["nc"]


def make_in_maps(x, Wq, Wk, Wv, Wo, betas):
    x = np.asarray(x, np.float32)
    Wq = np.asarray(Wq, np.float32)
    Wk = np.asarray(Wk, np.float32)
    Wv = np.asarray(Wv, np.float32)
    Wo = np.asarray(Wo, np.float32)
    betas = np.asarray(betas, np.float32)

    xT = [np.ascontiguousarray(x[b].T) for b in range(B)]
    wo16 = np.ascontiguousarray(Wo.astype(np.float16))
    t = np.arange(SEG)
    mask = np.where(t[:, None] <= t[None, :], 0.0, MASKVAL).astype(np.float32)
    beta_full = 1.0 / (1.0 + np.exp(-betas))  # (1,H,1,DV)

    in_maps = []
    for c in range(NCORE):
        b, q = divmod(c, HPC)
        sl = slice(CH * q, CH * (q + 1))
        in_maps.append({
            "xT": xT[b],
            "wq": np.ascontiguousarray(Wq[:, sl]),
            "wk": np.ascontiguousarray(Wk[:, sl]),
            "wv": np.ascontiguousarray(Wv[:, sl]),
            "wo": wo16,
            "mask": mask,
            "beta": np.ascontiguousarray(
                beta_full[0, HPC * q:HPC * (q + 1), 0, :].T),
        })
    return in_maps


def gather(results):
    out = np.empty((B, NSEG, 512, D), np.float32)
    for c in range(NCORE):
        b, q = divmod(c, HPC)
        out[b, :, 128 * q:128 * (q + 1), :] = results[c]["out"]
    return out.reshape(B, S, D)


def kernel(x, Wq, Wk, Wv, Wo, betas):
    from concourse import bass2jax
    nc = get_module()
    in_maps = make_in_maps(x, Wq, Wk, Wv, Wo, betas)
    results = bass2jax.run_bass_via_pjrt(nc, in_maps, n_cores=NCORE)
    return gather(results)



# revision 2
# speedup vs baseline: 1.6185x; 1.6185x over previous
"""CompressiveMemory (Infini-attention style) Trainium2 Bass kernel.

Sharding: 8 cores = batch(2) x head-quad(4). Core c handles batch b=c//4 and
heads [4*(c%4), 4*(c%4)+4). The reference's `att.reshape(B, SEG, H*DV)` is a
torch-style view of the contiguous (B,H,SEG,DV) array, so segment-output row
r = h*32 + s//16 depends on ONE head only: each core produces rows
[128*(c%4), 128*(c%4)+128) of every 512-row segment block, and the host
gather is a pure concat (no cross-core reduction).

DMA traffic is the bottleneck in this environment (cores' DMA serializes),
so all streamed tensors are f16 and the projection weights are loaded into
SBUF once per call instead of once per segment: ~35MB/core vs 145MB/core.

Per-core per-segment compute (all layouts chosen so no activation transposes
are needed):
  qT/kT = W^T @ xT-slice        [chan, tok]   (f16 matmuls, f32 PSUM)
  v     = xT-slice^T @ Wv       [tok, chan]
  per head: scoresT = kT^T qT; e = exp((scoresT+mask)/sqrt(dk));
            den = ones^T e; U = v^T e; sigma_q/k = elu()+1;
            R = mem^T sigma_q; zden = z^T sigma_q;
            attT = U/den + beta*(R/zden - U/den)
            retz = sigma_kT^T [mem|z]; ndelta = ret/kvden - v;
            mem -= sigma_k_nat^T ndelta; z += rowsum(sigma_kT)
  out rows = scrambled-view(attT) @ Wo   (f16 matmuls, full Wo resident)
"""
import numpy as np

import concourse.bass as bass
import concourse.mybir as mybir
import concourse.tile as tile
from concourse import bacc
from concourse.masks import make_identity

B, S, D = 2, 4096, 2048
H, DK, DV = 16, 128, 128
SEG = 512
NSEG = S // SEG
NCORE = 8
HPC = 4                      # heads per core
CH = HPC * DK                # 512 per-core q/k/v channels
SCALE = float(DK) ** -0.5
MASKVAL = -4.0e5             # pre-scale additive mask; exp((s+M)*SCALE) -> 0

f32 = mybir.dt.float32
f32r = mybir.dt.float32r
f16 = mybir.dt.float16
ALU = mybir.AluOpType
ACTF = mybir.ActivationFunctionType
AXIS = mybir.AxisListType

_MODULE_CACHE = {}


def _build_module():
    nc = bacc.Bacc("TRN2", target_bir_lowering=False, debug=False,
                   num_devices=NCORE)
    xT_d = nc.dram_tensor("xT", [D, S], f16, kind="ExternalInput")
    wq_d = nc.dram_tensor("wq", [D, CH], f16, kind="ExternalInput")
    wk_d = nc.dram_tensor("wk", [D, CH], f16, kind="ExternalInput")
    wv_d = nc.dram_tensor("wv", [D, CH], f16, kind="ExternalInput")
    wo_d = nc.dram_tensor("wo", [D, D], f16, kind="ExternalInput")
    mask_d = nc.dram_tensor("mask", [SEG, SEG], f32, kind="ExternalInput")
    beta_d = nc.dram_tensor("beta", [DV, HPC], f32, kind="ExternalInput")
    out_d = nc.dram_tensor("out", [NSEG, 128, D], f16, kind="ExternalOutput")

    with tile.TileContext(nc) as tc:
        _body(nc, tc, xT_d, wq_d, wk_d, wv_d, wo_d, mask_d, beta_d, out_d)
    nc.compile()
    return nc


def _body(nc, tc, xT_d, wq_d, wk_d, wv_d, wo_d, mask_d, beta_d, out_d):
    with (
        tc.tile_pool(name="statics", bufs=1) as st,
        tc.tile_pool(name="xt", bufs=16) as xt_pool,
        tc.tile_pool(name="qkv", bufs=4) as qkv_pool,
        tc.tile_pool(name="sig", bufs=2) as sig_pool,
        tc.tile_pool(name="tmp", bufs=6) as tmp_pool,
        tc.tile_pool(name="exps", bufs=4) as exps_pool,
        tc.tile_pool(name="attp", bufs=2) as att_pool,
        tc.tile_pool(name="ndp", bufs=4) as nd_pool,
        tc.tile_pool(name="rvec", bufs=3) as rv_pool,
        tc.tile_pool(name="tiny", bufs=6) as tiny_pool,
        tc.tile_pool(name="outs", bufs=4) as out_pool,
        tc.tile_pool(name="mm", bufs=5, space=bass.MemorySpace.PSUM) as pp,
        tc.tile_pool(name="aux", bufs=3, space=bass.MemorySpace.PSUM) as pa,
    ):
        # ---- statics ----
        wo_sb = st.tile([128, 16 * D], f16, tag="wo")
        for j in range(16):
            nc.sync.dma_start(out=wo_sb[:, j * D:(j + 1) * D],
                              in_=wo_d[j * 128:(j + 1) * 128, :])
        # resident projection weights: 16 d-chunks x [128, CH] f16 each
        wq_sb = st.tile([128, 16 * CH], f16, tag="wq")
        wk_sb = st.tile([128, 16 * CH], f16, tag="wk")
        wv_sb = st.tile([128, 16 * CH], f16, tag="wv")
        for i in range(16):
            nc.sync.dma_start(out=wq_sb[:, i * CH:(i + 1) * CH],
                              in_=wq_d[i * 128:(i + 1) * 128, :])
            nc.sync.dma_start(out=wk_sb[:, i * CH:(i + 1) * CH],
                              in_=wk_d[i * 128:(i + 1) * 128, :])
            nc.sync.dma_start(out=wv_sb[:, i * CH:(i + 1) * CH],
                              in_=wv_d[i * 128:(i + 1) * 128, :])
        mask_sb = st.tile([128, 4 * SEG], f32, tag="mask")
        for c4 in range(4):
            nc.sync.dma_start(out=mask_sb[:, c4 * SEG:(c4 + 1) * SEG],
                              in_=mask_d[c4 * 128:(c4 + 1) * 128, :])
        beta_sb = st.tile([DV, HPC], f32, tag="beta")
        nc.sync.dma_start(out=beta_sb[:], in_=beta_d[:])
        ident = st.tile([128, 128], f32, tag="ident")
        make_identity(nc, ident[:])
        # f32r/f16 cannot be memset directly: stage in f32, copy (copy rounds).
        ones32f = st.tile([128, 32], f32, tag="ones32f")
        nc.vector.memset(ones32f[:], 1.0)
        ones32 = st.tile([128, 32], f16, tag="ones32")
        nc.vector.tensor_copy(ones32[:], ones32f[:])
        invf = st.tile([32, 128], f32, tag="invf")
        nc.vector.memset(invf[:], 1.0 / 32.0)
        inv32 = st.tile([32, 128], f32r, tag="inv32")
        nc.vector.tensor_copy(inv32[:], invf[:])
        # per-head memory state [dk, mem(128) | z(1) | zero-pad(127)]
        mzf = st.tile([128, 256], f32, tag="mzf")
        nc.vector.memset(mzf[:], 0.0)
        nc.vector.memset(mzf[:, 128:129], 1.0 / DK)
        mem_sb = []
        for h in range(HPC):
            m = st.tile([128, 256], f32r, tag=f"mem{h}")
            nc.vector.tensor_copy(m[:], mzf[:])
            mem_sb.append(m)

        # ---- main loop ----
        for seg in range(NSEG):
            # xT slice tiles [d-tile 128, SEG] f16
            xt = []
            for i in range(16):
                t = xt_pool.tile([128, SEG], f16, tag="xt")
                nc.sync.dma_start(
                    out=t[:], in_=xT_d[i * 128:(i + 1) * 128,
                                       seg * SEG:(seg + 1) * SEG])
                xt.append(t)

            def proj_T(w_sb, dtag):
                """qT/kT: [chan, tok] in 4 chunks of [128, SEG]."""
                dests = []
                ps = [pp.tile([128, SEG], f32, tag="mm", name=f"ps_{dtag}{c}")
                      for c in range(4)]
                for i in range(16):
                    for c in range(4):
                        nc.tensor.matmul(ps[c][:],
                                         w_sb[:, i * CH + c * 128:
                                              i * CH + (c + 1) * 128],
                                         xt[i][:],
                                         start=(i == 0), stop=(i == 15))
                for c in range(4):
                    dst = qkv_pool.tile([128, SEG], f16, tag=dtag)
                    nc.vector.tensor_copy(dst[:], ps[c][:])
                    dests.append(dst)
                return dests

            def proj_N(w_sb, dtag):
                """v: [tok, chan] in 4 token-chunks of [128, CH]."""
                dests = []
                ps = [pp.tile([128, CH], f32, tag="mm", name=f"ps_{dtag}{c}")
                      for c in range(4)]
                for i in range(16):
                    for c in range(4):
                        nc.tensor.matmul(ps[c][:],
                                         xt[i][:, c * 128:(c + 1) * 128],
                                         w_sb[:, i * CH:(i + 1) * CH],
                                         start=(i == 0), stop=(i == 15))
                for c in range(4):
                    dst = qkv_pool.tile([128, CH], f16, tag=dtag)
                    nc.scalar.copy(dst[:], ps[c][:])
                    dests.append(dst)
                return dests

            qT = proj_T(wq_sb, "qT")
            kT = proj_T(wk_sb, "kT")
            v = proj_N(wv_sb, "v")

            attT = att_pool.tile([128, HPC * SEG], f16, tag="attT")

            for h in range(HPC):
                memh = mem_sb[h]

                def elu1(src, dtag, accum=None):
                    """sigma = elu(src)+1 = exp(min(src,0)) + relu(src)."""
                    mn = tmp_pool.tile([128, SEG], f32, tag="tmp")
                    nc.vector.tensor_scalar_min(mn[:], src[:], 0.0)
                    e = tmp_pool.tile([128, SEG], f32, tag="tmp")
                    nc.scalar.activation(e[:], mn[:], ACTF.Exp)
                    r = tmp_pool.tile([128, SEG], f32, tag="tmp")
                    nc.scalar.activation(r[:], src[:], ACTF.Relu)
                    out = sig_pool.tile([128, SEG], f32r, tag=dtag)
                    nc.vector.tensor_add(out[:], e[:], r[:])
                    return out

                sgq = elu1(qT[h], "sgq")
                sgk = elu1(kT[h], "sgk")
                # z increment = rowsum of sigma_kT over tokens
                zsum = tiny_pool.tile([128, 1], f32, tag="zsum")
                nc.vector.reduce_sum(zsum[:], sgk[:], axis=AXIS.X)
                # sigma_k natural layout via PE transpose
                signat = sig_pool.tile([128, SEG], f32r, tag="signat")
                for c4 in range(4):
                    pt = pa.tile([128, 128], f32, tag="aux")
                    nc.tensor.transpose(pt[:],
                                        sgk[:, c4 * 128:(c4 + 1) * 128].bitcast(f32),
                                        ident[:])
                    nc.vector.tensor_copy(
                        signat[:, c4 * 128:(c4 + 1) * 128], pt[:])

                # scoresT chunks -> exp((S+mask)*SCALE)
                es = []
                for c4 in range(4):
                    psc = pp.tile([128, SEG], f32, tag="mm")
                    nc.tensor.matmul(psc[:],
                                     kT[h][:, c4 * 128:(c4 + 1) * 128],
                                     qT[h][:])
                    nc.vector.tensor_tensor(
                        psc[:], psc[:],
                        mask_sb[:, c4 * SEG:(c4 + 1) * SEG], op=ALU.add)
                    e = exps_pool.tile([128, SEG], f16, tag="exps")
                    nc.scalar.activation(e[:], psc[:], ACTF.Exp, scale=SCALE)
                    es.append(e)

                pden = pa.tile([32, SEG], f32, tag="aux")
                for c4 in range(4):
                    nc.tensor.matmul(pden[:], ones32[:], es[c4][:],
                                     start=(c4 == 0), stop=(c4 == 3))
                pU = pp.tile([128, SEG], f32, tag="mm")
                for c4 in range(4):
                    nc.tensor.matmul(pU[:],
                                     v[c4][:, h * 128:(h + 1) * 128],
                                     es[c4][:],
                                     start=(c4 == 0), stop=(c4 == 3))
                pR = pp.tile([128, SEG], f32, tag="mm")
                nc.tensor.matmul(pR[:], memh[:, 0:128], sgq[:])
                # zden rows: replicate z into 32 cols, then M=32 matmul
                zrep = tiny_pool.tile([128, 32], f32r, tag="zrep")
                nc.vector.tensor_scalar_mul(zrep[:], ones32f[:],
                                            memh[:, 128:129].bitcast(f32))
                pzd = pa.tile([32, SEG], f32, tag="aux")
                nc.tensor.matmul(pzd[:], zrep[:], sgq[:])

                rden = rv_pool.tile([32, SEG], f32r, tag="rvec")
                rzden = rv_pool.tile([32, SEG], f32r, tag="rvec")
                with nc.allow_low_precision(reason="fp32r for PE broadcast"):
                    nc.vector.reciprocal(rden[:], pden[:])
                    nc.vector.reciprocal(rzden[:], pzd[:])
                pbd = pp.tile([128, SEG], f32, tag="mm")
                nc.tensor.matmul(pbd[:], inv32[:], rden[:])
                pbz = pp.tile([128, SEG], f32, tag="mm")
                nc.tensor.matmul(pbz[:], inv32[:], rzden[:])

                # DVE cannot read two PSUM operands in one op: stage the
                # broadcasts through SBUF on the scalar engine first.
                bd = tmp_pool.tile([128, SEG], f32, tag="tmp")
                nc.scalar.copy(bd[:], pbd[:])
                bz = tmp_pool.tile([128, SEG], f32, tag="tmp")
                nc.scalar.copy(bz[:], pbz[:])
                t1 = tmp_pool.tile([128, SEG], f32, tag="tmp")
                nc.vector.tensor_tensor(t1[:], pU[:], bd[:], op=ALU.mult)
                t2 = tmp_pool.tile([128, SEG], f32, tag="tmp")
                nc.vector.tensor_tensor(t2[:], pR[:], bz[:], op=ALU.mult)
                nc.vector.tensor_sub(t2[:], t2[:], t1[:])
                nc.vector.scalar_tensor_tensor(
                    attT[:, h * SEG:(h + 1) * SEG],
                    t2[:], beta_sb[:, h:h + 1], t1[:],
                    op0=ALU.mult, op1=ALU.add)

                # ---- memory update (delta rule) ----
                pmu = pa.tile([128, 128], f32, tag="aux")
                for c4 in range(4):
                    prz = pa.tile([128, 256], f32, tag="aux")
                    nc.tensor.matmul(prz[:],
                                     sgk[:, c4 * 128:(c4 + 1) * 128],
                                     memh[:])
                    rk = tiny_pool.tile([128, 1], f32, tag="rk")
                    nc.vector.reciprocal(rk[:], prz[:, 128:129])
                    nd = nd_pool.tile([128, 128], f32r, tag="nd")
                    nc.vector.scalar_tensor_tensor(
                        nd[:], prz[:, 0:128], rk[:],
                        v[c4][:, h * 128:(h + 1) * 128],
                        op0=ALU.mult, op1=ALU.subtract)
                    nc.tensor.matmul(pmu[:],
                                     signat[:, c4 * 128:(c4 + 1) * 128],
                                     nd[:],
                                     start=(c4 == 0), stop=(c4 == 3))
                nc.vector.tensor_sub(memh[:, 0:128], memh[:, 0:128], pmu[:])
                nc.vector.tensor_tensor(memh[:, 128:129], memh[:, 128:129],
                                        zsum[:], op=ALU.add)

            # ---- output projection (torch-view scramble baked into the AP) ----
            # row r = h*32+g <- attT column h*512 + 16*g + j, contracted over
            # (j, v) against Wo rows j*128+v.
            attv = attT[:].rearrange("p (h g j) -> p h g j", h=HPC, g=32, j=16)
            for o in range(4):
                po = pp.tile([128, 512], f32, tag="mm")
                for j in range(16):
                    nc.tensor.matmul(
                        po[:], attv[:, :, :, j],
                        wo_sb[:, j * D + o * 512: j * D + o * 512 + 512],
                        start=(j == 0), stop=(j == 15))
                osb = out_pool.tile([128, 512], f16, tag="outs")
                if o % 2 == 0:
                    nc.scalar.copy(osb[:], po[:])
                else:
                    nc.vector.tensor_copy(osb[:], po[:])
                nc.sync.dma_start(out=out_d[seg, :, o * 512:(o + 1) * 512],
                                  in_=osb[:])


def get_module():
    if "nc" not in _MODULE_CACHE:
        _MODULE_CACHE["nc"] = _build_module()
    return _MODULE_CACHE["nc"]


def make_in_maps(x, Wq, Wk, Wv, Wo, betas):
    x = np.asarray(x, np.float32)
    Wq = np.asarray(Wq, np.float32)
    Wk = np.asarray(Wk, np.float32)
    Wv = np.asarray(Wv, np.float32)
    Wo = np.asarray(Wo, np.float32)
    betas = np.asarray(betas, np.float32)

    xT = [np.ascontiguousarray(x[b].T).astype(np.float16) for b in range(B)]
    wo16 = np.ascontiguousarray(Wo.astype(np.float16))
    t = np.arange(SEG)
    mask = np.where(t[:, None] <= t[None, :], 0.0, MASKVAL).astype(np.float32)
    beta_full = 1.0 / (1.0 + np.exp(-betas))  # (1,H,1,DV)

    in_maps = []
    for c in range(NCORE):
        b, q = divmod(c, HPC)
        sl = slice(CH * q, CH * (q + 1))
        in_maps.append({
            "xT": xT[b],
            "wq": np.ascontiguousarray(Wq[:, sl].astype(np.float16)),
            "wk": np.ascontiguousarray(Wk[:, sl].astype(np.float16)),
            "wv": np.ascontiguousarray(Wv[:, sl].astype(np.float16)),
            "wo": wo16,
            "mask": mask,
            "beta": np.ascontiguousarray(
                beta_full[0, HPC * q:HPC * (q + 1), 0, :].T),
        })
    return in_maps


def gather(results):
    out = np.empty((B, NSEG, 512, D), np.float32)
    for c in range(NCORE):
        b, q = divmod(c, HPC)
        out[b, :, 128 * q:128 * (q + 1), :] = results[c]["out"].astype(
            np.float32)
    return out.reshape(B, S, D)


def kernel(x, Wq, Wk, Wv, Wo, betas):
    from concourse import bass2jax
    nc = get_module()
    in_maps = make_in_maps(x, Wq, Wk, Wv, Wo, betas)
    results = bass2jax.run_bass_via_pjrt(nc, in_maps, n_cores=NCORE)
    return gather(results)


# revision 5
# speedup vs baseline: 4.0235x; 2.4859x over previous
"""CompressiveMemory (Infini-attention style) Trainium2 Bass kernel.

Sharding: 8 cores = batch(2) x head-quad(4). Core c handles batch b=c//4 and
heads [4*(c%4), 4*(c%4)+4). The reference's `att.reshape(B, SEG, H*DV)` is a
torch-style view of the contiguous (B,H,SEG,DV) array, so segment-output row
r = h*32 + s//16 depends on ONE head only: each core produces rows
[128*(c%4), 128*(c%4)+128) of every 512-row segment block, and the host
gather is a pure concat (no cross-core reduction).

DMA traffic is the bottleneck in this environment (cores' DMA serializes),
so all streamed tensors are f16, the projection weights live in SBUF for the
whole call, DMAs are batched into one instruction per tensor (per segment),
Wo streams on the scalar queue overlapping segment-0 compute, and the causal
mask is generated on-device via affine_select on the exp output.
~31MB/core vs 145MB/core in the naive version.

Per-core per-segment compute (all layouts chosen so no activation transposes
are needed):
  qT/kT = W^T @ xT-slice        [chan, tok]   (f16 matmuls, f32 PSUM)
  v     = xT-slice^T @ Wv       [tok, chan]
  per head: scoresT = kT^T qT; e = causal_mask(exp(scoresT/sqrt(dk)));
            den = ones^T e; U = v^T e; sigma_q/k = elu()+1;
            R = mem^T sigma_q; zden = z^T sigma_q;
            attT = U/den + beta*(R/zden - U/den)
            retz = sigma_kT^T [mem|z]; ndelta = ret/kvden - v;
            mem -= sigma_k_nat^T ndelta; z += rowsum(sigma_kT)
  out rows = scrambled-view(attT) @ Wo   (f16 matmuls, full Wo resident)
"""
import numpy as np

import concourse.bass as bass
import concourse.mybir as mybir
import concourse.tile as tile
from concourse import bacc
from concourse.masks import make_identity

B, S, D = 2, 4096, 2048
H, DK, DV = 16, 128, 128
SEG = 512
NSEG = S // SEG
NCORE = 8
HPC = 4                      # heads per core
CH = HPC * DK                # 512 per-core q/k/v channels
SCALE = float(DK) ** -0.5

f32 = mybir.dt.float32
f32r = mybir.dt.float32r
f16 = mybir.dt.float16
ALU = mybir.AluOpType
ACTF = mybir.ActivationFunctionType
AXIS = mybir.AxisListType

_MODULE_CACHE = {}


def _build_module():
    nc = bacc.Bacc("TRN2", target_bir_lowering=False, debug=False,
                   num_devices=NCORE)
    xT_d = nc.dram_tensor("xT", [D, S], f16, kind="ExternalInput")
    wq_d = nc.dram_tensor("wq", [D, CH], f16, kind="ExternalInput")
    wk_d = nc.dram_tensor("wk", [D, CH], f16, kind="ExternalInput")
    wv_d = nc.dram_tensor("wv", [D, CH], f16, kind="ExternalInput")
    wo_d = nc.dram_tensor("wo", [D, D], f16, kind="ExternalInput")
    beta_d = nc.dram_tensor("beta", [DV, HPC], f32, kind="ExternalInput")
    out_d = nc.dram_tensor("out", [NSEG, 128, D], f16, kind="ExternalOutput")

    with tile.TileContext(nc) as tc:
        _body(nc, tc, xT_d, wq_d, wk_d, wv_d, wo_d, beta_d, out_d)
    nc.compile()
    return nc


def _body(nc, tc, xT_d, wq_d, wk_d, wv_d, wo_d, beta_d, out_d):
    with (
        tc.tile_pool(name="statics", bufs=1) as st,
        tc.tile_pool(name="xt", bufs=2) as xt_pool,
        tc.tile_pool(name="qkv", bufs=4) as qkv_pool,
        tc.tile_pool(name="sig", bufs=2) as sig_pool,
        tc.tile_pool(name="tmp", bufs=5) as tmp_pool,
        tc.tile_pool(name="exps", bufs=4) as exps_pool,
        tc.tile_pool(name="attp", bufs=2) as att_pool,
        tc.tile_pool(name="ndp", bufs=2) as nd_pool,
        tc.tile_pool(name="rvec", bufs=2) as rv_pool,
        tc.tile_pool(name="tiny", bufs=4) as tiny_pool,
        tc.tile_pool(name="outs", bufs=1) as out_pool,
        tc.tile_pool(name="mm", bufs=5, space=bass.MemorySpace.PSUM) as pp,
        tc.tile_pool(name="aux", bufs=3, space=bass.MemorySpace.PSUM) as pa,
    ):
        # ---- statics ----
        # projection weights first (segment 0 needs them immediately),
        # each as a single batched DMA on the sync queue.
        wq_sb = st.tile([128, 16 * CH], f16, tag="wq")
        wk_sb = st.tile([128, 16 * CH], f16, tag="wk")
        wv_sb = st.tile([128, 16 * CH], f16, tag="wv")
        for w_sb, w_d in ((wq_sb, wq_d), (wk_sb, wk_d), (wv_sb, wv_d)):
            nc.sync.dma_start(
                out=w_sb[:].rearrange("p (i c) -> p i c", i=16),
                in_=w_d.rearrange("(i p) c -> p i c", p=128))
        # Wo on the scalar queue: overlaps the sync queue's segment-0/1 x
        # loads; only needed at the first output projection.
        wo_sb = st.tile([128, 16 * D], f16, tag="wo")
        nc.scalar.dma_start(
            out=wo_sb[:].rearrange("p (i c) -> p i c", i=16),
            in_=wo_d.rearrange("(i p) c -> p i c", p=128))
        beta_sb = st.tile([DV, HPC], f32, tag="beta")
        nc.sync.dma_start(out=beta_sb[:], in_=beta_d[:])
        ident = st.tile([128, 128], f32, tag="ident")
        make_identity(nc, ident[:])
        # f32r/f16 cannot be memset directly: stage in f32, copy (copy rounds).
        ones32f = st.tile([128, 32], f32, tag="ones32f")
        nc.vector.memset(ones32f[:], 1.0)
        ones32 = st.tile([128, 32], f16, tag="ones32")
        nc.vector.tensor_copy(ones32[:], ones32f[:])
        invf = st.tile([32, 128], f32, tag="invf")
        nc.vector.memset(invf[:], 1.0 / 32.0)
        inv32 = st.tile([32, 128], f32r, tag="inv32")
        nc.vector.tensor_copy(inv32[:], invf[:])
        # per-head memory state [dk, mem(128) | z(1) | zero-pad(127)]
        mzf = st.tile([128, 256], f32, tag="mzf")
        nc.vector.memset(mzf[:], 0.0)
        nc.vector.memset(mzf[:, 128:129], 1.0 / DK)
        mem_sb = []
        for h in range(HPC):
            m = st.tile([128, 256], f32r, tag=f"mem{h}")
            nc.vector.tensor_copy(m[:], mzf[:])
            mem_sb.append(m)

        # ---- main loop ----
        for seg in range(NSEG):
            # one batched DMA for the whole [2048, SEG] xT slice:
            # xt[p, i*SEG + t] = xT[i*128 + p, seg*SEG + t]
            xt = xt_pool.tile([128, 16 * SEG], f16, tag="xt")
            nc.sync.dma_start(
                out=xt[:].rearrange("p (i t) -> p i t", i=16),
                in_=xT_d[:, seg * SEG:(seg + 1) * SEG]
                    .rearrange("(i p) t -> p i t", p=128))

            def proj_T(w_sb, dtag):
                """qT/kT: [chan, tok] in 4 chunks of [128, SEG]."""
                dests = []
                ps = [pp.tile([128, SEG], f32, tag="mm", name=f"ps_{dtag}{c}")
                      for c in range(4)]
                for i in range(16):
                    for c in range(4):
                        nc.tensor.matmul(ps[c][:],
                                         w_sb[:, i * CH + c * 128:
                                              i * CH + (c + 1) * 128],
                                         xt[:, i * SEG:(i + 1) * SEG],
                                         start=(i == 0), stop=(i == 15))
                for c in range(4):
                    dst = qkv_pool.tile([128, SEG], f16, tag=dtag)
                    nc.vector.tensor_copy(dst[:], ps[c][:])
                    dests.append(dst)
                return dests

            def proj_N(w_sb, dtag):
                """v: [tok, chan] in 4 token-chunks of [128, CH]."""
                dests = []
                ps = [pp.tile([128, CH], f32, tag="mm", name=f"ps_{dtag}{c}")
                      for c in range(4)]
                for i in range(16):
                    for c in range(4):
                        nc.tensor.matmul(ps[c][:],
                                         xt[:, i * SEG + c * 128:
                                            i * SEG + (c + 1) * 128],
                                         w_sb[:, i * CH:(i + 1) * CH],
                                         start=(i == 0), stop=(i == 15))
                for c in range(4):
                    dst = qkv_pool.tile([128, CH], f16, tag=dtag)
                    nc.scalar.copy(dst[:], ps[c][:])
                    dests.append(dst)
                return dests

            qT = proj_T(wq_sb, "qT")
            kT = proj_T(wk_sb, "kT")
            v = proj_N(wv_sb, "v")

            attT = att_pool.tile([128, HPC * SEG], f16, tag="attT")

            for h in range(HPC):
                memh = mem_sb[h]

                def elu1(src, dtag):
                    """sigma = elu(src)+1 = exp(min(src,0)) + relu(src)."""
                    mn = tmp_pool.tile([128, SEG], f32, tag="tmp")
                    nc.vector.tensor_scalar_min(mn[:], src[:], 0.0)
                    e = tmp_pool.tile([128, SEG], f32, tag="tmp")
                    nc.scalar.activation(e[:], mn[:], ACTF.Exp)
                    r = tmp_pool.tile([128, SEG], f32, tag="tmp")
                    nc.scalar.activation(r[:], src[:], ACTF.Relu)
                    out = sig_pool.tile([128, SEG], f32r, tag=dtag)
                    nc.vector.tensor_add(out[:], e[:], r[:])
                    return out

                sgq = elu1(qT[h], "sgq")
                sgk = elu1(kT[h], "sgk")
                # z increment = rowsum of sigma_kT over tokens
                zsum = tiny_pool.tile([128, 1], f32, tag="zsum")
                nc.vector.reduce_sum(zsum[:], sgk[:], axis=AXIS.X)
                # sigma_k natural layout via PE transpose
                signat = sig_pool.tile([128, SEG], f32r, tag="signat")
                for c4 in range(4):
                    pt = pa.tile([128, 128], f32, tag="aux")
                    nc.tensor.transpose(pt[:],
                                        sgk[:, c4 * 128:(c4 + 1) * 128].bitcast(f32),
                                        ident[:])
                    nc.vector.tensor_copy(
                        signat[:, c4 * 128:(c4 + 1) * 128], pt[:])

                # scoresT chunks -> exp(S*SCALE), causal-masked in place:
                # chunk c4 holds keys k=c4*128+p vs queries t; keep t >= k.
                es = []
                for c4 in range(4):
                    psc = pp.tile([128, SEG], f32, tag="mm")
                    nc.tensor.matmul(psc[:],
                                     kT[h][:, c4 * 128:(c4 + 1) * 128],
                                     qT[h][:])
                    e = exps_pool.tile([128, SEG], f16, tag="exps")
                    nc.scalar.activation(e[:], psc[:], ACTF.Exp, scale=SCALE)
                    nc.gpsimd.affine_select(
                        e[:], e[:], pattern=[[1, SEG]],
                        compare_op=ALU.is_ge, fill=0.0,
                        base=-c4 * 128, channel_multiplier=-1)
                    es.append(e)

                pden = pa.tile([32, SEG], f32, tag="aux")
                for c4 in range(4):
                    nc.tensor.matmul(pden[:], ones32[:], es[c4][:],
                                     start=(c4 == 0), stop=(c4 == 3))
                pU = pp.tile([128, SEG], f32, tag="mm")
                for c4 in range(4):
                    nc.tensor.matmul(pU[:],
                                     v[c4][:, h * 128:(h + 1) * 128],
                                     es[c4][:],
                                     start=(c4 == 0), stop=(c4 == 3))
                pR = pp.tile([128, SEG], f32, tag="mm")
                nc.tensor.matmul(pR[:], memh[:, 0:128], sgq[:])
                # zden rows: replicate z into 32 cols, then M=32 matmul
                zrep = tiny_pool.tile([128, 32], f32r, tag="zrep")
                nc.vector.tensor_scalar_mul(zrep[:], ones32f[:],
                                            memh[:, 128:129].bitcast(f32))
                pzd = pa.tile([32, SEG], f32, tag="aux")
                nc.tensor.matmul(pzd[:], zrep[:], sgq[:])

                rden = rv_pool.tile([32, SEG], f32r, tag="rvec")
                rzden = rv_pool.tile([32, SEG], f32r, tag="rvec")
                with nc.allow_low_precision(reason="fp32r for PE broadcast"):
                    nc.vector.reciprocal(rden[:], pden[:])
                    nc.vector.reciprocal(rzden[:], pzd[:])
                pbd = pp.tile([128, SEG], f32, tag="mm")
                nc.tensor.matmul(pbd[:], inv32[:], rden[:])
                pbz = pp.tile([128, SEG], f32, tag="mm")
                nc.tensor.matmul(pbz[:], inv32[:], rzden[:])

                # DVE cannot read two PSUM operands in one op: stage the
                # broadcasts through SBUF on the scalar engine first.
                bd = tmp_pool.tile([128, SEG], f32, tag="tmp")
                nc.scalar.copy(bd[:], pbd[:])
                bz = tmp_pool.tile([128, SEG], f32, tag="tmp")
                nc.scalar.copy(bz[:], pbz[:])
                t1 = tmp_pool.tile([128, SEG], f32, tag="tmp")
                nc.vector.tensor_tensor(t1[:], pU[:], bd[:], op=ALU.mult)
                t2 = tmp_pool.tile([128, SEG], f32, tag="tmp")
                nc.vector.tensor_tensor(t2[:], pR[:], bz[:], op=ALU.mult)
                nc.vector.tensor_sub(t2[:], t2[:], t1[:])
                nc.vector.scalar_tensor_tensor(
                    attT[:, h * SEG:(h + 1) * SEG],
                    t2[:], beta_sb[:, h:h + 1], t1[:],
                    op0=ALU.mult, op1=ALU.add)

                # ---- memory update (delta rule) ----
                pmu = pa.tile([128, 128], f32, tag="aux")
                for c4 in range(4):
                    prz = pa.tile([128, 256], f32, tag="aux")
                    nc.tensor.matmul(prz[:],
                                     sgk[:, c4 * 128:(c4 + 1) * 128],
                                     memh[:])
                    rk = tiny_pool.tile([128, 1], f32, tag="rk")
                    nc.vector.reciprocal(rk[:], prz[:, 128:129])
                    nd = nd_pool.tile([128, 128], f32r, tag="nd")
                    nc.vector.scalar_tensor_tensor(
                        nd[:], prz[:, 0:128], rk[:],
                        v[c4][:, h * 128:(h + 1) * 128],
                        op0=ALU.mult, op1=ALU.subtract)
                    nc.tensor.matmul(pmu[:],
                                     signat[:, c4 * 128:(c4 + 1) * 128],
                                     nd[:],
                                     start=(c4 == 0), stop=(c4 == 3))
                nc.vector.tensor_sub(memh[:, 0:128], memh[:, 0:128], pmu[:])
                nc.vector.tensor_tensor(memh[:, 128:129], memh[:, 128:129],
                                        zsum[:], op=ALU.add)

            # ---- output projection (torch-view scramble baked into the AP) ----
            # row r = h*32+g <- attT column h*512 + 16*g + j, contracted over
            # (j, v) against Wo rows j*128+v.
            attv = attT[:].rearrange("p (h g j) -> p h g j", h=HPC, g=32, j=16)
            osb = out_pool.tile([128, D], f16, tag="outs")
            for o in range(4):
                po = pp.tile([128, 512], f32, tag="mm")
                for j in range(16):
                    nc.tensor.matmul(
                        po[:], attv[:, :, :, j],
                        wo_sb[:, j * D + o * 512: j * D + o * 512 + 512],
                        start=(j == 0), stop=(j == 15))
                if o % 2 == 0:
                    nc.scalar.copy(osb[:, o * 512:(o + 1) * 512], po[:])
                else:
                    nc.vector.tensor_copy(osb[:, o * 512:(o + 1) * 512], po[:])
            nc.scalar.dma_start(out=out_d[seg], in_=osb[:])


def get_module():
    if "nc" not in _MODULE_CACHE:
        _MODULE_CACHE["nc"] = _build_module()
    return _MODULE_CACHE["nc"]


def make_in_maps(x, Wq, Wk, Wv, Wo, betas):
    x = np.asarray(x, np.float32)
    Wq = np.asarray(Wq, np.float32)
    Wk = np.asarray(Wk, np.float32)
    Wv = np.asarray(Wv, np.float32)
    Wo = np.asarray(Wo, np.float32)
    betas = np.asarray(betas, np.float32)

    xT = [np.ascontiguousarray(x[b].T).astype(np.float16) for b in range(B)]
    wo16 = np.ascontiguousarray(Wo.astype(np.float16))
    beta_full = 1.0 / (1.0 + np.exp(-betas))  # (1,H,1,DV)

    in_maps = []
    for c in range(NCORE):
        b, q = divmod(c, HPC)
        sl = slice(CH * q, CH * (q + 1))
        in_maps.append({
            "xT": xT[b],
            "wq": np.ascontiguousarray(Wq[:, sl].astype(np.float16)),
            "wk": np.ascontiguousarray(Wk[:, sl].astype(np.float16)),
            "wv": np.ascontiguousarray(Wv[:, sl].astype(np.float16)),
            "wo": wo16,
            "beta": np.ascontiguousarray(
                beta_full[0, HPC * q:HPC * (q + 1), 0, :].T),
        })
    return in_maps


def gather(results):
    out = np.empty((B, NSEG, 512, D), np.float32)
    for c in range(NCORE):
        b, q = divmod(c, HPC)
        out[b, :, 128 * q:128 * (q + 1), :] = results[c]["out"].astype(
            np.float32)
    return out.reshape(B, S, D)


def kernel(x, Wq, Wk, Wv, Wo, betas):
    from concourse import bass2jax
    nc = get_module()
    in_maps = make_in_maps(x, Wq, Wk, Wv, Wo, betas)
    results = bass2jax.run_bass_via_pjrt(nc, in_maps, n_cores=NCORE)
    return gather(results)


# revision 6
# speedup vs baseline: 4.0251x; 1.0004x over previous
"""CompressiveMemory kernel, single-core variant.

The axon environment serializes all cores' DMA through one ~10.5GB/s pipe
and caches repeated reads of the same DRAM region (~95% discount), so the
metric to minimize is UNIQUE bytes. One core reading x once (32MB f16),
the weights once (32MB f16) and writing out (32MB f16) beats any multi-core
split, which duplicates x and Wo per core.

Structure: 2 sequential head-octet phases; per phase the octet's Wq/Wk/Wv
column-slices (12MB f16) are SBUF-resident, Wo streams per (b,seg) with
cached repeats, x streams per (b,seg) (phase B re-reads are cached).
Out rows of a segment block split by head (torch-view scramble), so each
phase writes rows [256*ph, 256*ph+256) of every segment independently.
"""
import numpy as np

import concourse.bass as bass
import concourse.mybir as mybir
import concourse.tile as tile
from concourse import bacc
from concourse.masks import make_identity

B, S, D = 2, 4096, 2048
H, DK, DV = 16, 128, 128
SEG = 512
NSEG = S // SEG
NCORE = 1
HPP = 8                      # heads per phase
CH = HPP * DK                # 1024 per-phase q/k/v channels
NPH = H // HPP               # 2 phases
SCALE = float(DK) ** -0.5

f32 = mybir.dt.float32
f32r = mybir.dt.float32r
f16 = mybir.dt.float16
ALU = mybir.AluOpType
ACTF = mybir.ActivationFunctionType
AXIS = mybir.AxisListType

_MODULE_CACHE = {}


def _build_module():
    nc = bacc.Bacc("TRN2", target_bir_lowering=False, debug=False,
                   num_devices=NCORE)
    # host-pretiled layouts for long DMA lines:
    # xT[b, seg, p, i*SEG+t] = x[b, seg*SEG+t, i*128+p]   (16KB/partition)
    # wq[ph, p, i*CH+c] = Wq[i*128+p, ph*CH+c]            (32KB/partition)
    xT_d = nc.dram_tensor("xT", [B, NSEG, 128, 16 * SEG], f16,
                          kind="ExternalInput")
    wq_d = nc.dram_tensor("wq", [NPH, 128, 16 * CH], f16,
                          kind="ExternalInput")
    wk_d = nc.dram_tensor("wk", [NPH, 128, 16 * CH], f16,
                          kind="ExternalInput")
    wv_d = nc.dram_tensor("wv", [NPH, 128, 16 * CH], f16,
                          kind="ExternalInput")
    wo_d = nc.dram_tensor("wo", [D, D], f16, kind="ExternalInput")
    beta_d = nc.dram_tensor("beta", [DV, H], f32, kind="ExternalInput")
    out_d = nc.dram_tensor("out", [B, NSEG, 512, D], f16,
                           kind="ExternalOutput")

    with tile.TileContext(nc) as tc:
        _body(nc, tc, xT_d, wq_d, wk_d, wv_d, wo_d, beta_d, out_d)
    nc.compile()
    return nc


def _body(nc, tc, xT_d, wq_d, wk_d, wv_d, wo_d, beta_d, out_d):
    from contextlib import ExitStack
    with ExitStack() as stack:
        ep = stack.enter_context
        st = ep(tc.tile_pool(name="statics", bufs=1))
        w_pool = ep(tc.tile_pool(name="wres", bufs=1))
        mem_pool = ep(tc.tile_pool(name="mems", bufs=1))
        xt_pool = ep(tc.tile_pool(name="xt", bufs=4))
        qkv_pool = ep(tc.tile_pool(name="qkv", bufs=8))
        v_pool = ep(tc.tile_pool(name="vpool", bufs=4))
        wo_pool = ep(tc.tile_pool(name="wo", bufs=3))
        sig_pool = ep(tc.tile_pool(name="sig", bufs=1))
        tmp_pool = ep(tc.tile_pool(name="tmp", bufs=4))
        exps_pool = ep(tc.tile_pool(name="exps", bufs=4))
        att_pool = ep(tc.tile_pool(name="attp", bufs=2))
        nd_pool = ep(tc.tile_pool(name="ndp", bufs=2))
        rv_pool = ep(tc.tile_pool(name="rvec", bufs=2))
        tiny_pool = ep(tc.tile_pool(name="tiny", bufs=4))
        out_pool = ep(tc.tile_pool(name="outs", bufs=3))
        pp = ep(tc.tile_pool(name="mm", bufs=5, space=bass.MemorySpace.PSUM))
        pa = ep(tc.tile_pool(name="aux", bufs=3, space=bass.MemorySpace.PSUM))
        # ---- global statics ----
        beta_sb = st.tile([DV, H], f32, tag="beta")
        nc.sync.dma_start(out=beta_sb[:], in_=beta_d[:])
        ident = st.tile([128, 128], f32, tag="ident")
        make_identity(nc, ident[:])
        ones32f = st.tile([128, 32], f32, tag="ones32f")
        nc.vector.memset(ones32f[:], 1.0)
        ones32 = st.tile([128, 32], f16, tag="ones32")
        nc.vector.tensor_copy(ones32[:], ones32f[:])
        invf = st.tile([32, 128], f32, tag="invf")
        nc.vector.memset(invf[:], 1.0 / 32.0)
        inv32 = st.tile([32, 128], f32r, tag="inv32")
        nc.vector.tensor_copy(inv32[:], invf[:])
        mzf = st.tile([128, 256], f32, tag="mzf")
        nc.vector.memset(mzf[:], 0.0)
        nc.vector.memset(mzf[:, 128:129], 1.0 / DK)

        for ph in range(NPH):
            # ---- phase weights: this octet's column slice, resident ----
            wq_sb = w_pool.tile([128, 16 * CH], f16, tag="wq")
            wk_sb = w_pool.tile([128, 16 * CH], f16, tag="wk")
            wv_sb = w_pool.tile([128, 16 * CH], f16, tag="wv")
            for w_sb, w_d in ((wq_sb, wq_d), (wk_sb, wk_d), (wv_sb, wv_d)):
                nc.sync.dma_start(out=w_sb[:], in_=w_d[ph])
            # ---- memory states for this phase: (batch, head) ----
            mem_sb = {}
            for b in range(B):
                for h in range(HPP):
                    m = mem_pool.tile([128, 256], f32r, tag=f"mem{b}_{h}")
                    nc.vector.tensor_copy(m[:], mzf[:])
                    mem_sb[(b, h)] = m

            for seg in range(NSEG):
                attTs = []
                for b in range(B):
                    # 4 quad-chunk tiles; DRAM lines are 4KB (pretiled host
                    # layout), 512 descriptor lines per (b, seg).
                    xts = []
                    for q in range(4):
                        t = xt_pool.tile([128, 4 * SEG], f16, tag="xt")
                        nc.sync.dma_start(
                            out=t[:],
                            in_=xT_d[b, seg, :, q * 4 * SEG:(q + 1) * 4 * SEG])
                        xts.append(t)


                    def proj_T(w_sb, dtag):
                        """qT/kT: [chan, tok], 8 chunks, 2 PSUM waves of 4."""
                        dests = []
                        for wave in range(2):
                            ps = [pp.tile([128, SEG], f32, tag="mm",
                                          name=f"ps_{dtag}{wave}{c}")
                                  for c in range(4)]
                            for i in range(16):
                                xti = xts[i // 4][:, (i % 4) * SEG:
                                                  (i % 4 + 1) * SEG]
                                for c in range(4):
                                    cc = wave * 4 + c
                                    nc.tensor.matmul(
                                        ps[c][:],
                                        w_sb[:, i * CH + cc * 128:
                                             i * CH + (cc + 1) * 128],
                                        xti,
                                        start=(i == 0), stop=(i == 15))
                            for c in range(4):
                                dst = qkv_pool.tile([128, SEG], f16, tag=dtag)
                                nc.vector.tensor_copy(dst[:], ps[c][:])
                                dests.append(dst)
                        return dests

                    def proj_N(w_sb, dtag):
                        """v: [tok, chan]: 4 tok-chunks x [128, CH] f16,
                        2 PSUM waves; each wave = 2 tok-chunks x 2
                        col-half accumulators of [128, 512]."""
                        dests = [v_pool.tile([128, CH], f16, tag=dtag,
                                             name=f"v{c}")
                                 for c in range(4)]
                        for wave in range(2):
                            ps = [pp.tile([128, 512], f32, tag="mm",
                                          name=f"ps_{dtag}{wave}{k}")
                                  for k in range(4)]
                            for i in range(16):
                                for c in range(2):
                                    cc = wave * 2 + c
                                    xtic = xts[i // 4][
                                        :, (i % 4) * SEG + cc * 128:
                                        (i % 4) * SEG + (cc + 1) * 128]
                                    for half in range(2):
                                        nc.tensor.matmul(
                                            ps[c * 2 + half][:],
                                            xtic,
                                            w_sb[:, i * CH + half * 512:
                                                 i * CH + half * 512 + 512],
                                            start=(i == 0), stop=(i == 15))
                            for c in range(2):
                                cc = wave * 2 + c
                                for half in range(2):
                                    nc.scalar.copy(
                                        dests[cc][:, half * 512:
                                                  half * 512 + 512],
                                        ps[c * 2 + half][:])
                        return dests

                    qT = proj_T(wq_sb, "qT")
                    kT = proj_T(wk_sb, "kT")
                    v = proj_N(wv_sb, "v")

                    attT = att_pool.tile([128, HPP * SEG], f16, tag="attT")
                    attTs.append(attT)

                    for h in range(HPP):
                        memh = mem_sb[(b, h)]

                        def elu1(src, dtag):
                            mn = tmp_pool.tile([128, SEG], f32, tag="tmp")
                            nc.vector.tensor_scalar_min(mn[:], src[:], 0.0)
                            e = tmp_pool.tile([128, SEG], f32, tag="tmp")
                            nc.scalar.activation(e[:], mn[:], ACTF.Exp)
                            r = tmp_pool.tile([128, SEG], f32, tag="tmp")
                            nc.scalar.activation(r[:], src[:], ACTF.Relu)
                            out = sig_pool.tile([128, SEG], f32r, tag=dtag)
                            nc.vector.tensor_add(out[:], e[:], r[:])
                            return out

                        sgq = elu1(qT[h], "sgq")
                        sgk = elu1(kT[h], "sgk")
                        zsum = tiny_pool.tile([128, 1], f32, tag="zsum")
                        nc.vector.reduce_sum(zsum[:], sgk[:], axis=AXIS.X)
                        signat = sig_pool.tile([128, SEG], f32r, tag="signat")
                        for c4 in range(4):
                            pt = pa.tile([128, 128], f32, tag="aux")
                            nc.tensor.transpose(
                                pt[:],
                                sgk[:, c4 * 128:(c4 + 1) * 128].bitcast(f32),
                                ident[:])
                            nc.vector.tensor_copy(
                                signat[:, c4 * 128:(c4 + 1) * 128], pt[:])

                        es = []
                        for c4 in range(4):
                            psc = pp.tile([128, SEG], f32, tag="mm")
                            nc.tensor.matmul(psc[:],
                                             kT[h][:, c4 * 128:(c4 + 1) * 128],
                                             qT[h][:])
                            e = exps_pool.tile([128, SEG], f16, tag="exps")
                            nc.scalar.activation(e[:], psc[:], ACTF.Exp,
                                                 scale=SCALE)
                            nc.gpsimd.affine_select(
                                e[:], e[:], pattern=[[1, SEG]],
                                compare_op=ALU.is_ge, fill=0.0,
                                base=-c4 * 128, channel_multiplier=-1)
                            es.append(e)

                        pden = pa.tile([32, SEG], f32, tag="aux")
                        for c4 in range(4):
                            nc.tensor.matmul(pden[:], ones32[:], es[c4][:],
                                             start=(c4 == 0), stop=(c4 == 3))
                        pU = pp.tile([128, SEG], f32, tag="mm")
                        for c4 in range(4):
                            nc.tensor.matmul(pU[:],
                                             v[c4][:, h * 128:(h + 1) * 128],
                                             es[c4][:],
                                             start=(c4 == 0), stop=(c4 == 3))
                        pR = pp.tile([128, SEG], f32, tag="mm")
                        nc.tensor.matmul(pR[:], memh[:, 0:128], sgq[:])
                        zrep = tiny_pool.tile([128, 32], f32r, tag="zrep")
                        nc.vector.tensor_scalar_mul(
                            zrep[:], ones32f[:],
                            memh[:, 128:129].bitcast(f32))
                        pzd = pa.tile([32, SEG], f32, tag="aux")
                        nc.tensor.matmul(pzd[:], zrep[:], sgq[:])

                        rden = rv_pool.tile([32, SEG], f32r, tag="rvec")
                        rzden = rv_pool.tile([32, SEG], f32r, tag="rvec")
                        with nc.allow_low_precision(
                                reason="fp32r for PE broadcast"):
                            nc.vector.reciprocal(rden[:], pden[:])
                            nc.vector.reciprocal(rzden[:], pzd[:])
                        pbd = pp.tile([128, SEG], f32, tag="mm")
                        nc.tensor.matmul(pbd[:], inv32[:], rden[:])
                        pbz = pp.tile([128, SEG], f32, tag="mm")
                        nc.tensor.matmul(pbz[:], inv32[:], rzden[:])

                        bd = tmp_pool.tile([128, SEG], f32, tag="tmp")
                        nc.scalar.copy(bd[:], pbd[:])
                        bz = tmp_pool.tile([128, SEG], f32, tag="tmp")
                        nc.scalar.copy(bz[:], pbz[:])
                        t1 = tmp_pool.tile([128, SEG], f32, tag="tmp")
                        nc.vector.tensor_tensor(t1[:], pU[:], bd[:],
                                                op=ALU.mult)
                        t2 = tmp_pool.tile([128, SEG], f32, tag="tmp")
                        nc.vector.tensor_tensor(t2[:], pR[:], bz[:],
                                                op=ALU.mult)
                        nc.vector.tensor_sub(t2[:], t2[:], t1[:])
                        nc.vector.scalar_tensor_tensor(
                            attT[:, h * SEG:(h + 1) * SEG],
                            t2[:], beta_sb[:, ph * HPP + h:ph * HPP + h + 1],
                            t1[:], op0=ALU.mult, op1=ALU.add)

                        pmu = pa.tile([128, 128], f32, tag="aux")
                        for c4 in range(4):
                            prz = pa.tile([128, 256], f32, tag="aux")
                            nc.tensor.matmul(prz[:],
                                             sgk[:, c4 * 128:(c4 + 1) * 128],
                                             memh[:])
                            rk = tiny_pool.tile([128, 1], f32, tag="rk")
                            nc.vector.reciprocal(rk[:], prz[:, 128:129])
                            nd = nd_pool.tile([128, 128], f32r, tag="nd")
                            nc.vector.scalar_tensor_tensor(
                                nd[:], prz[:, 0:128], rk[:],
                                v[c4][:, h * 128:(h + 1) * 128],
                                op0=ALU.mult, op1=ALU.subtract)
                            nc.tensor.matmul(pmu[:],
                                             signat[:, c4 * 128:(c4 + 1) * 128],
                                             nd[:],
                                             start=(c4 == 0), stop=(c4 == 3))
                        nc.vector.tensor_sub(memh[:, 0:128], memh[:, 0:128],
                                             pmu[:])
                        nc.vector.tensor_tensor(memh[:, 128:129],
                                                memh[:, 128:129],
                                                zsum[:], op=ALU.add)

                # ---- joint output projection for both batches: Wo is read
                # once per (phase, seg, col-half) instead of once per batch.
                # 8 PSUM accumulators = 2 batches x 2 rowblocks x 2 o-chunks.
                attv = [
                    attTs[b][:, rb * 4 * SEG:(rb + 1) * 4 * SEG]
                    .rearrange("p (h g j) -> p h g j", h=4, g=32, j=16)
                    for b in range(B) for rb in range(2)]
                for half in range(2):
                    po = [pp.tile([128, 512], f32, tag="mm",
                                  name=f"po{k}") for k in range(5)]
                    po += [pa.tile([128, 512], f32, tag="aux",
                                   name=f"po{5 + k}") for k in range(3)]
                    for j in range(16):
                        wot = wo_pool.tile([128, D // 2], f16, tag="wo")
                        nc.scalar.dma_start(
                            out=wot[:],
                            in_=wo_d[j * 128:(j + 1) * 128,
                                     half * 1024:(half + 1) * 1024])
                        for br in range(4):
                            for oh in range(2):
                                nc.tensor.matmul(
                                    po[br * 2 + oh][:],
                                    attv[br][:, :, :, j],
                                    wot[:, oh * 512:(oh + 1) * 512],
                                    start=(j == 0), stop=(j == 15))
                    for br in range(4):
                        b_, rb = divmod(br, 2)
                        osb = out_pool.tile([128, D // 2], f16, tag="outs")
                        nc.scalar.copy(osb[:, 0:512], po[br * 2][:])
                        nc.vector.tensor_copy(osb[:, 512:1024],
                                              po[br * 2 + 1][:])
                        r0 = 256 * ph + 128 * rb
                        nc.scalar.dma_start(
                            out=out_d[b_, seg, r0:r0 + 128,
                                      half * 1024:(half + 1) * 1024],
                            in_=osb[:])


def get_module():
    if "nc" not in _MODULE_CACHE:
        _MODULE_CACHE["nc"] = _build_module()
    return _MODULE_CACHE["nc"]


def _tile_w(W):
    """[D, D] -> [NPH, 128, 16*CH]: w[ph, p, i*CH+c] = W[i*128+p, ph*CH+c]."""
    w = np.asarray(W, np.float32).astype(np.float16)
    w = w.reshape(16, 128, NPH, CH).transpose(2, 1, 0, 3)
    return np.ascontiguousarray(w.reshape(NPH, 128, 16 * CH))


def make_in_maps(x, Wq, Wk, Wv, Wo, betas):
    x = np.asarray(x, np.float32).astype(np.float16)
    # xT[b, seg, p, i*SEG+t] = x[b, seg*SEG+t, i*128+p]  (16KB DMA lines)
    xT = x.reshape(B, NSEG, SEG, 16, 128).transpose(0, 1, 4, 3, 2)
    xT = np.ascontiguousarray(xT.reshape(B, NSEG, 128, 16 * SEG))
    beta_full = 1.0 / (1.0 + np.exp(-np.asarray(betas, np.float32)))
    return [{
        "xT": xT,
        "wq": _tile_w(Wq),
        "wk": _tile_w(Wk),
        "wv": _tile_w(Wv),
        "wo": np.ascontiguousarray(np.asarray(Wo, np.float32)
                                   .astype(np.float16)),
        "beta": np.ascontiguousarray(beta_full[0, :, 0, :].T),
    }]


def gather(results):
    out = results[0]["out"].astype(np.float32)  # [B, NSEG, 512, D]
    return out.reshape(B, S, D)


def kernel(x, Wq, Wk, Wv, Wo, betas):
    from concourse import bass2jax
    nc = get_module()
    in_maps = make_in_maps(x, Wq, Wk, Wv, Wo, betas)
    results = bass2jax.run_bass_via_pjrt(nc, in_maps, n_cores=NCORE)
    return gather(results)


# revision 7
# speedup vs baseline: 4.0355x; 1.0026x over previous
"""CompressiveMemory kernel, single-core variant.

The axon environment serializes all cores' DMA through one ~10.5GB/s pipe
and caches repeated reads of the same DRAM region (~95% discount), so the
metric to minimize is UNIQUE bytes. One core reading x once (32MB f16),
the weights once (32MB f16) and writing out (32MB f16) beats any multi-core
split, which duplicates x and Wo per core.

Structure: 2 sequential head-octet phases; per phase the octet's Wq/Wk/Wv
column-slices (12MB f16) are SBUF-resident, Wo streams per (b,seg) with
cached repeats, x streams per (b,seg) (phase B re-reads are cached).
Out rows of a segment block split by head (torch-view scramble), so each
phase writes rows [256*ph, 256*ph+256) of every segment independently.
"""
import numpy as np

import concourse.bass as bass
import concourse.mybir as mybir
import concourse.tile as tile
from concourse import bacc
from concourse.masks import make_identity

B, S, D = 2, 4096, 2048
H, DK, DV = 16, 128, 128
SEG = 512
NSEG = S // SEG
NCORE = 1
HPP = 8                      # heads per phase
CH = HPP * DK                # 1024 per-phase q/k/v channels
NPH = H // HPP               # 2 phases
SCALE = float(DK) ** -0.5

f32 = mybir.dt.float32
f32r = mybir.dt.float32r
f16 = mybir.dt.float16
ALU = mybir.AluOpType
ACTF = mybir.ActivationFunctionType
AXIS = mybir.AxisListType

_MODULE_CACHE = {}


def _build_module():
    nc = bacc.Bacc("TRN2", target_bir_lowering=False, debug=False,
                   num_devices=NCORE)
    # host-pretiled layouts for long DMA lines:
    # xT[b, seg, p, i*SEG+t] = x[b, seg*SEG+t, i*128+p]   (16KB/partition)
    # wq[ph, p, i*CH+c] = Wq[i*128+p, ph*CH+c]            (32KB/partition)
    xT_d = nc.dram_tensor("xT", [B, NSEG, 128, 16 * SEG], f16,
                          kind="ExternalInput")
    wq_d = nc.dram_tensor("wq", [NPH, 128, 16 * CH], f16,
                          kind="ExternalInput")
    wk_d = nc.dram_tensor("wk", [NPH, 128, 16 * CH], f16,
                          kind="ExternalInput")
    wv_d = nc.dram_tensor("wv", [NPH, 128, 16 * CH], f16,
                          kind="ExternalInput")
    wo_d = nc.dram_tensor("wo", [D, D], f16, kind="ExternalInput")
    beta_d = nc.dram_tensor("beta", [DV, H], f32, kind="ExternalInput")
    out_d = nc.dram_tensor("out", [B, NSEG, 512, D], f16,
                           kind="ExternalOutput")

    with tile.TileContext(nc) as tc:
        _body(nc, tc, xT_d, wq_d, wk_d, wv_d, wo_d, beta_d, out_d)
    nc.compile()
    return nc


def _body(nc, tc, xT_d, wq_d, wk_d, wv_d, wo_d, beta_d, out_d):
    from contextlib import ExitStack
    with ExitStack() as stack:
        ep = stack.enter_context
        st = ep(tc.tile_pool(name="statics", bufs=1))
        w_pool = ep(tc.tile_pool(name="wres", bufs=1))
        mem_pool = ep(tc.tile_pool(name="mems", bufs=1))
        xt_pool = ep(tc.tile_pool(name="xt", bufs=4))
        qkv_pool = ep(tc.tile_pool(name="qkv", bufs=8))
        v_pool = ep(tc.tile_pool(name="vpool", bufs=4))
        wo_pool = ep(tc.tile_pool(name="wo", bufs=3))
        sig_pool = ep(tc.tile_pool(name="sig", bufs=2))
        tmp_pool = ep(tc.tile_pool(name="tmp", bufs=4))
        exps_pool = ep(tc.tile_pool(name="exps", bufs=4))
        att_pool = ep(tc.tile_pool(name="attp", bufs=2))
        nd_pool = ep(tc.tile_pool(name="ndp", bufs=2))
        rv_pool = ep(tc.tile_pool(name="rvec", bufs=2))
        tiny_pool = ep(tc.tile_pool(name="tiny", bufs=4))
        out_pool = ep(tc.tile_pool(name="outs", bufs=3))
        pp = ep(tc.tile_pool(name="mm", bufs=5, space=bass.MemorySpace.PSUM))
        pa = ep(tc.tile_pool(name="aux", bufs=3, space=bass.MemorySpace.PSUM))
        # ---- global statics ----
        beta_sb = st.tile([DV, H], f32, tag="beta")
        nc.sync.dma_start(out=beta_sb[:], in_=beta_d[:])
        ident = st.tile([128, 128], f32, tag="ident")
        make_identity(nc, ident[:])
        ident16 = st.tile([128, 128], f16, tag="ident16")
        nc.vector.tensor_copy(ident16[:], ident[:])
        ones32f = st.tile([128, 32], f32, tag="ones32f")
        nc.vector.memset(ones32f[:], 1.0)
        ones32 = st.tile([128, 32], f16, tag="ones32")
        nc.vector.tensor_copy(ones32[:], ones32f[:])
        invf = st.tile([32, 128], f32, tag="invf")
        nc.vector.memset(invf[:], 1.0 / 32.0)
        inv32 = st.tile([32, 128], f32r, tag="inv32")
        nc.vector.tensor_copy(inv32[:], invf[:])
        mzf = st.tile([128, 256], f32, tag="mzf")
        nc.vector.memset(mzf[:], 0.0)
        nc.vector.memset(mzf[:, 128:129], 1.0 / DK)

        for ph in range(NPH):
            # ---- phase weights: this octet's column slice, resident ----
            wq_sb = w_pool.tile([128, 16 * CH], f16, tag="wq")
            wk_sb = w_pool.tile([128, 16 * CH], f16, tag="wk")
            wv_sb = w_pool.tile([128, 16 * CH], f16, tag="wv")
            for w_sb, w_d in ((wq_sb, wq_d), (wk_sb, wk_d), (wv_sb, wv_d)):
                nc.sync.dma_start(out=w_sb[:], in_=w_d[ph])
            # ---- memory states for this phase: (batch, head) ----
            mem_sb = {}
            for b in range(B):
                for h in range(HPP):
                    m = mem_pool.tile([128, 256], f16, tag=f"mem{b}_{h}")
                    nc.vector.tensor_copy(m[:], mzf[:])
                    zf = mem_pool.tile([128, 1], f32, tag=f"z{b}_{h}")
                    nc.vector.memset(zf[:], 1.0 / DK)
                    mem_sb[(b, h)] = (m, zf)

            for seg in range(NSEG):
                attTs = []
                for b in range(B):
                    # 4 quad-chunk tiles; DRAM lines are 4KB (pretiled host
                    # layout), 512 descriptor lines per (b, seg).
                    xts = []
                    for q in range(4):
                        t = xt_pool.tile([128, 4 * SEG], f16, tag="xt")
                        nc.sync.dma_start(
                            out=t[:],
                            in_=xT_d[b, seg, :, q * 4 * SEG:(q + 1) * 4 * SEG])
                        xts.append(t)


                    def proj_T(w_sb, dtag):
                        """qT/kT: [chan, tok], 8 chunks, 2 PSUM waves of 4."""
                        dests = []
                        for wave in range(2):
                            ps = [pp.tile([128, SEG], f32, tag="mm",
                                          name=f"ps_{dtag}{wave}{c}")
                                  for c in range(4)]
                            for i in range(16):
                                xti = xts[i // 4][:, (i % 4) * SEG:
                                                  (i % 4 + 1) * SEG]
                                for c in range(4):
                                    cc = wave * 4 + c
                                    nc.tensor.matmul(
                                        ps[c][:],
                                        w_sb[:, i * CH + cc * 128:
                                             i * CH + (cc + 1) * 128],
                                        xti,
                                        start=(i == 0), stop=(i == 15))
                            for c in range(4):
                                dst = qkv_pool.tile([128, SEG], f16, tag=dtag)
                                nc.vector.tensor_copy(dst[:], ps[c][:])
                                dests.append(dst)
                        return dests

                    def proj_N(w_sb, dtag):
                        """v: [tok, chan]: 4 tok-chunks x [128, CH] f16,
                        2 PSUM waves; each wave = 2 tok-chunks x 2
                        col-half accumulators of [128, 512]."""
                        dests = [v_pool.tile([128, CH], f16, tag=dtag,
                                             name=f"v{c}")
                                 for c in range(4)]
                        for wave in range(2):
                            ps = [pp.tile([128, 512], f32, tag="mm",
                                          name=f"ps_{dtag}{wave}{k}")
                                  for k in range(4)]
                            for i in range(16):
                                for c in range(2):
                                    cc = wave * 2 + c
                                    xtic = xts[i // 4][
                                        :, (i % 4) * SEG + cc * 128:
                                        (i % 4) * SEG + (cc + 1) * 128]
                                    for half in range(2):
                                        nc.tensor.matmul(
                                            ps[c * 2 + half][:],
                                            xtic,
                                            w_sb[:, i * CH + half * 512:
                                                 i * CH + half * 512 + 512],
                                            start=(i == 0), stop=(i == 15))
                            for c in range(2):
                                cc = wave * 2 + c
                                for half in range(2):
                                    nc.scalar.copy(
                                        dests[cc][:, half * 512:
                                                  half * 512 + 512],
                                        ps[c * 2 + half][:])
                        return dests

                    qT = proj_T(wq_sb, "qT")
                    kT = proj_T(wk_sb, "kT")
                    v = proj_N(wv_sb, "v")

                    attT = att_pool.tile([128, HPP * SEG], f16, tag="attT")
                    attTs.append(attT)

                    for h in range(HPP):
                        memh, zf32 = mem_sb[(b, h)]

                        def elu1(src, dtag):
                            mn = tmp_pool.tile([128, SEG], f32, tag="tmp")
                            nc.vector.tensor_scalar_min(mn[:], src[:], 0.0)
                            e = tmp_pool.tile([128, SEG], f32, tag="tmp")
                            nc.scalar.activation(e[:], mn[:], ACTF.Exp)
                            r = tmp_pool.tile([128, SEG], f32, tag="tmp")
                            nc.scalar.activation(r[:], src[:], ACTF.Relu)
                            out = sig_pool.tile([128, SEG], f16, tag=dtag)
                            nc.vector.tensor_add(out[:], e[:], r[:])
                            return out

                        sgq = elu1(qT[h], "sgq")
                        sgk = elu1(kT[h], "sgk")
                        zsum = tiny_pool.tile([128, 1], f32, tag="zsum")
                        nc.vector.reduce_sum(zsum[:], sgk[:], axis=AXIS.X)
                        signat = sig_pool.tile([128, SEG], f16, tag="signat")
                        for c4 in range(4):
                            pt = pa.tile([128, 128], f16, tag="aux")
                            nc.tensor.transpose(
                                pt[:],
                                sgk[:, c4 * 128:(c4 + 1) * 128],
                                ident16[:])
                            nc.vector.tensor_copy(
                                signat[:, c4 * 128:(c4 + 1) * 128], pt[:])

                        es = []
                        for c4 in range(4):
                            psc = pp.tile([128, SEG], f32, tag="mm")
                            nc.tensor.matmul(psc[:],
                                             kT[h][:, c4 * 128:(c4 + 1) * 128],
                                             qT[h][:])
                            e = exps_pool.tile([128, SEG], f16, tag="exps")
                            nc.scalar.activation(e[:], psc[:], ACTF.Exp,
                                                 scale=SCALE)
                            nc.gpsimd.affine_select(
                                e[:], e[:], pattern=[[1, SEG]],
                                compare_op=ALU.is_ge, fill=0.0,
                                base=-c4 * 128, channel_multiplier=-1)
                            es.append(e)

                        pden = pa.tile([32, SEG], f32, tag="aux")
                        for c4 in range(4):
                            nc.tensor.matmul(pden[:], ones32[:], es[c4][:],
                                             start=(c4 == 0), stop=(c4 == 3))
                        pU = pp.tile([128, SEG], f32, tag="mm")
                        for c4 in range(4):
                            nc.tensor.matmul(pU[:],
                                             v[c4][:, h * 128:(h + 1) * 128],
                                             es[c4][:],
                                             start=(c4 == 0), stop=(c4 == 3))
                        pR = pp.tile([128, SEG], f32, tag="mm")
                        nc.tensor.matmul(pR[:], memh[:, 0:128], sgq[:])
                        zrep = tiny_pool.tile([128, 32], f16, tag="zrep")
                        nc.vector.tensor_scalar_mul(
                            zrep[:], ones32f[:], zf32[:, 0:1])
                        pzd = pa.tile([32, SEG], f32, tag="aux")
                        nc.tensor.matmul(pzd[:], zrep[:], sgq[:])

                        rden = rv_pool.tile([32, SEG], f32r, tag="rvec")
                        rzden = rv_pool.tile([32, SEG], f32r, tag="rvec")
                        with nc.allow_low_precision(
                                reason="fp32r for PE broadcast"):
                            nc.vector.reciprocal(rden[:], pden[:])
                            nc.vector.reciprocal(rzden[:], pzd[:])
                        pbd = pp.tile([128, SEG], f32, tag="mm")
                        nc.tensor.matmul(pbd[:], inv32[:], rden[:])
                        pbz = pp.tile([128, SEG], f32, tag="mm")
                        nc.tensor.matmul(pbz[:], inv32[:], rzden[:])

                        bd = tmp_pool.tile([128, SEG], f32, tag="tmp")
                        nc.scalar.copy(bd[:], pbd[:])
                        bz = tmp_pool.tile([128, SEG], f32, tag="tmp")
                        nc.scalar.copy(bz[:], pbz[:])
                        t1 = tmp_pool.tile([128, SEG], f32, tag="tmp")
                        nc.vector.tensor_tensor(t1[:], pU[:], bd[:],
                                                op=ALU.mult)
                        t2 = tmp_pool.tile([128, SEG], f32, tag="tmp")
                        nc.vector.tensor_tensor(t2[:], pR[:], bz[:],
                                                op=ALU.mult)
                        nc.vector.tensor_sub(t2[:], t2[:], t1[:])
                        nc.vector.scalar_tensor_tensor(
                            attT[:, h * SEG:(h + 1) * SEG],
                            t2[:], beta_sb[:, ph * HPP + h:ph * HPP + h + 1],
                            t1[:], op0=ALU.mult, op1=ALU.add)

                        pmu = pa.tile([128, 128], f32, tag="aux")
                        for c4 in range(4):
                            prz = pa.tile([128, 256], f32, tag="aux")
                            nc.tensor.matmul(prz[:],
                                             sgk[:, c4 * 128:(c4 + 1) * 128],
                                             memh[:])
                            rk = tiny_pool.tile([128, 1], f32, tag="rk")
                            nc.vector.reciprocal(rk[:], prz[:, 128:129])
                            nd = nd_pool.tile([128, 128], f16, tag="nd")
                            nc.vector.scalar_tensor_tensor(
                                nd[:], prz[:, 0:128], rk[:],
                                v[c4][:, h * 128:(h + 1) * 128],
                                op0=ALU.mult, op1=ALU.subtract)
                            nc.tensor.matmul(pmu[:],
                                             signat[:, c4 * 128:(c4 + 1) * 128],
                                             nd[:],
                                             start=(c4 == 0), stop=(c4 == 3))
                        nc.vector.tensor_sub(memh[:, 0:128], memh[:, 0:128],
                                             pmu[:])
                        nc.vector.tensor_tensor(memh[:, 128:129],
                                                memh[:, 128:129],
                                                zsum[:], op=ALU.add)
                        nc.vector.tensor_tensor(zf32[:], zf32[:],
                                                zsum[:], op=ALU.add)

                # ---- joint output projection for both batches: Wo is read
                # once per (phase, seg, col-half) instead of once per batch.
                # 8 PSUM accumulators = 2 batches x 2 rowblocks x 2 o-chunks.
                attv = [
                    attTs[b][:, rb * 4 * SEG:(rb + 1) * 4 * SEG]
                    .rearrange("p (h g j) -> p h g j", h=4, g=32, j=16)
                    for b in range(B) for rb in range(2)]
                for half in range(2):
                    po = [pp.tile([128, 512], f32, tag="mm",
                                  name=f"po{k}") for k in range(5)]
                    po += [pa.tile([128, 512], f32, tag="aux",
                                   name=f"po{5 + k}") for k in range(3)]
                    for j in range(16):
                        wot = wo_pool.tile([128, D // 2], f16, tag="wo")
                        nc.scalar.dma_start(
                            out=wot[:],
                            in_=wo_d[j * 128:(j + 1) * 128,
                                     half * 1024:(half + 1) * 1024])
                        for br in range(4):
                            for oh in range(2):
                                nc.tensor.matmul(
                                    po[br * 2 + oh][:],
                                    attv[br][:, :, :, j],
                                    wot[:, oh * 512:(oh + 1) * 512],
                                    start=(j == 0), stop=(j == 15))
                    for br in range(4):
                        b_, rb = divmod(br, 2)
                        osb = out_pool.tile([128, D // 2], f16, tag="outs")
                        nc.scalar.copy(osb[:, 0:512], po[br * 2][:])
                        nc.vector.tensor_copy(osb[:, 512:1024],
                                              po[br * 2 + 1][:])
                        r0 = 256 * ph + 128 * rb
                        nc.scalar.dma_start(
                            out=out_d[b_, seg, r0:r0 + 128,
                                      half * 1024:(half + 1) * 1024],
                            in_=osb[:])


def get_module():
    if "nc" not in _MODULE_CACHE:
        _MODULE_CACHE["nc"] = _build_module()
    return _MODULE_CACHE["nc"]


def _tile_w(W):
    """[D, D] -> [NPH, 128, 16*CH]: w[ph, p, i*CH+c] = W[i*128+p, ph*CH+c]."""
    w = np.asarray(W, np.float32).astype(np.float16)
    w = w.reshape(16, 128, NPH, CH).transpose(2, 1, 0, 3)
    return np.ascontiguousarray(w.reshape(NPH, 128, 16 * CH))


def make_in_maps(x, Wq, Wk, Wv, Wo, betas):
    x = np.asarray(x, np.float32).astype(np.float16)
    # xT[b, seg, p, i*SEG+t] = x[b, seg*SEG+t, i*128+p]  (16KB DMA lines)
    xT = x.reshape(B, NSEG, SEG, 16, 128).transpose(0, 1, 4, 3, 2)
    xT = np.ascontiguousarray(xT.reshape(B, NSEG, 128, 16 * SEG))
    beta_full = 1.0 / (1.0 + np.exp(-np.asarray(betas, np.float32)))
    return [{
        "xT": xT,
        "wq": _tile_w(Wq),
        "wk": _tile_w(Wk),
        "wv": _tile_w(Wv),
        "wo": np.ascontiguousarray(np.asarray(Wo, np.float32)
                                   .astype(np.float16)),
        "beta": np.ascontiguousarray(beta_full[0, :, 0, :].T),
    }]


def gather(results):
    out = results[0]["out"].astype(np.float32)  # [B, NSEG, 512, D]
    return out.reshape(B, S, D)


def kernel(x, Wq, Wk, Wv, Wo, betas):
    from concourse import bass2jax
    nc = get_module()
    in_maps = make_in_maps(x, Wq, Wk, Wv, Wo, betas)
    results = bass2jax.run_bass_via_pjrt(nc, in_maps, n_cores=NCORE)
    return gather(results)


# revision 8
# speedup vs baseline: 4.0772x; 1.0103x over previous
"""CompressiveMemory kernel, single-core variant.

The axon environment serializes all cores' DMA through one ~10.5GB/s pipe
and caches repeated reads of the same DRAM region (~95% discount), so the
metric to minimize is UNIQUE bytes. One core reading x once (32MB f16),
the weights once (32MB f16) and writing out (32MB f16) beats any multi-core
split, which duplicates x and Wo per core.

Structure: 2 sequential head-octet phases; per phase the octet's Wq/Wk/Wv
column-slices (12MB f16) are SBUF-resident, Wo streams per (b,seg) with
cached repeats, x streams per (b,seg) (phase B re-reads are cached).
Out rows of a segment block split by head (torch-view scramble), so each
phase writes rows [256*ph, 256*ph+256) of every segment independently.
"""
import numpy as np

import concourse.bass as bass
import concourse.mybir as mybir
import concourse.tile as tile
from concourse import bacc
from concourse.masks import make_identity

B, S, D = 2, 4096, 2048
H, DK, DV = 16, 128, 128
SEG = 512
NSEG = S // SEG
NCORE = 1
HPP = 8                      # heads per phase
CH = HPP * DK                # 1024 per-phase q/k/v channels
NPH = H // HPP               # 2 phases
SCALE = float(DK) ** -0.5

f32 = mybir.dt.float32
f32r = mybir.dt.float32r
f16 = mybir.dt.float16
ALU = mybir.AluOpType
ACTF = mybir.ActivationFunctionType
AXIS = mybir.AxisListType

_MODULE_CACHE = {}


def _build_module():
    nc = bacc.Bacc("TRN2", target_bir_lowering=False, debug=False,
                   num_devices=NCORE)
    # host-pretiled layouts for long DMA lines:
    # xT[b, seg, p, i*SEG+t] = x[b, seg*SEG+t, i*128+p]   (16KB/partition)
    # wq[ph, p, i*CH+c] = Wq[i*128+p, ph*CH+c]            (32KB/partition)
    xT_d = nc.dram_tensor("xT", [B, NSEG, 128, 16 * SEG], f16,
                          kind="ExternalInput")
    wq_d = nc.dram_tensor("wq", [NPH, 128, 16 * CH], f16,
                          kind="ExternalInput")
    wk_d = nc.dram_tensor("wk", [NPH, 128, 16 * CH], f16,
                          kind="ExternalInput")
    wv_d = nc.dram_tensor("wv", [NPH, 128, 16 * CH], f16,
                          kind="ExternalInput")
    wo_d = nc.dram_tensor("wo", [D, D], f16, kind="ExternalInput")
    beta_d = nc.dram_tensor("beta", [DV, H], f32, kind="ExternalInput")
    out_d = nc.dram_tensor("out", [B, NSEG, 512, D], f16,
                           kind="ExternalOutput")

    with tile.TileContext(nc) as tc:
        _body(nc, tc, xT_d, wq_d, wk_d, wv_d, wo_d, beta_d, out_d)
    nc.compile()
    return nc


def _body(nc, tc, xT_d, wq_d, wk_d, wv_d, wo_d, beta_d, out_d):
    from contextlib import ExitStack
    with ExitStack() as stack:
        ep = stack.enter_context
        st = ep(tc.tile_pool(name="statics", bufs=1))
        w_pool = ep(tc.tile_pool(name="wres", bufs=1))
        mem_pool = ep(tc.tile_pool(name="mems", bufs=1))
        xt_pool = ep(tc.tile_pool(name="xt", bufs=4))
        qkv_pool = ep(tc.tile_pool(name="qkv", bufs=8))
        v_pool = ep(tc.tile_pool(name="vpool", bufs=4))
        wo_pool = ep(tc.tile_pool(name="wo", bufs=5))
        sig_pool = ep(tc.tile_pool(name="sig", bufs=2))
        tmp_pool = ep(tc.tile_pool(name="tmp", bufs=4))
        exps_pool = ep(tc.tile_pool(name="exps", bufs=4))
        att_pool = ep(tc.tile_pool(name="attp", bufs=2))
        nd_pool = ep(tc.tile_pool(name="ndp", bufs=2))
        rv_pool = ep(tc.tile_pool(name="rvec", bufs=2))
        tiny_pool = ep(tc.tile_pool(name="tiny", bufs=4))
        out_pool = ep(tc.tile_pool(name="outs", bufs=4))
        pp = ep(tc.tile_pool(name="mm", bufs=5, space=bass.MemorySpace.PSUM))
        pa = ep(tc.tile_pool(name="aux", bufs=3, space=bass.MemorySpace.PSUM))
        # ---- global statics ----
        beta_sb = st.tile([DV, H], f32, tag="beta")
        nc.sync.dma_start(out=beta_sb[:], in_=beta_d[:])
        ident = st.tile([128, 128], f32, tag="ident")
        make_identity(nc, ident[:])
        ident16 = st.tile([128, 128], f16, tag="ident16")
        nc.vector.tensor_copy(ident16[:], ident[:])
        ones32f = st.tile([128, 32], f32, tag="ones32f")
        nc.vector.memset(ones32f[:], 1.0)
        ones32 = st.tile([128, 32], f16, tag="ones32")
        nc.vector.tensor_copy(ones32[:], ones32f[:])
        invf = st.tile([32, 128], f32, tag="invf")
        nc.vector.memset(invf[:], 1.0 / 32.0)
        inv32 = st.tile([32, 128], f32r, tag="inv32")
        nc.vector.tensor_copy(inv32[:], invf[:])
        mzf = st.tile([128, 256], f32, tag="mzf")
        nc.vector.memset(mzf[:], 0.0)
        nc.vector.memset(mzf[:, 128:129], 1.0 / DK)

        for ph in range(NPH):
            # ---- phase weights: this octet's column slice, resident ----
            wq_sb = w_pool.tile([128, 16 * CH], f16, tag="wq")
            wk_sb = w_pool.tile([128, 16 * CH], f16, tag="wk")
            wv_sb = w_pool.tile([128, 16 * CH], f16, tag="wv")
            for w_sb, w_d in ((wq_sb, wq_d), (wk_sb, wk_d), (wv_sb, wv_d)):
                nc.sync.dma_start(out=w_sb[:], in_=w_d[ph])
            # ---- memory states for this phase: (batch, head) ----
            mem_sb = {}
            for b in range(B):
                for h in range(HPP):
                    m = mem_pool.tile([128, 256], f16, tag=f"mem{b}_{h}")
                    nc.vector.tensor_copy(m[:], mzf[:])
                    zf = mem_pool.tile([128, 1], f32, tag=f"z{b}_{h}")
                    nc.vector.memset(zf[:], 1.0 / DK)
                    mem_sb[(b, h)] = (m, zf)

            for seg in range(NSEG):
                attTs = []
                for b in range(B):
                    # 4 quad-chunk tiles; DRAM lines are 4KB (pretiled host
                    # layout), 512 descriptor lines per (b, seg).
                    xts = []
                    for q in range(4):
                        t = xt_pool.tile([128, 4 * SEG], f16, tag="xt")
                        nc.sync.dma_start(
                            out=t[:],
                            in_=xT_d[b, seg, :, q * 4 * SEG:(q + 1) * 4 * SEG])
                        xts.append(t)


                    def proj_T(w_sb, dtag):
                        """qT/kT: [chan, tok], 8 chunks, 2 PSUM waves of 4."""
                        dests = []
                        for wave in range(2):
                            ps = [pp.tile([128, SEG], f32, tag="mm",
                                          name=f"ps_{dtag}{wave}{c}")
                                  for c in range(4)]
                            for i in range(16):
                                xti = xts[i // 4][:, (i % 4) * SEG:
                                                  (i % 4 + 1) * SEG]
                                for c in range(4):
                                    cc = wave * 4 + c
                                    nc.tensor.matmul(
                                        ps[c][:],
                                        w_sb[:, i * CH + cc * 128:
                                             i * CH + (cc + 1) * 128],
                                        xti,
                                        start=(i == 0), stop=(i == 15))
                            for c in range(4):
                                dst = qkv_pool.tile([128, SEG], f16, tag=dtag)
                                nc.vector.tensor_copy(dst[:], ps[c][:])
                                dests.append(dst)
                        return dests

                    def proj_N(w_sb, dtag):
                        """v: [tok, chan]: 4 tok-chunks x [128, CH] f16,
                        2 PSUM waves; each wave = 2 tok-chunks x 2
                        col-half accumulators of [128, 512]."""
                        dests = [v_pool.tile([128, CH], f16, tag=dtag,
                                             name=f"v{c}")
                                 for c in range(4)]
                        for wave in range(2):
                            ps = [pp.tile([128, 512], f32, tag="mm",
                                          name=f"ps_{dtag}{wave}{k}")
                                  for k in range(4)]
                            for i in range(16):
                                for c in range(2):
                                    cc = wave * 2 + c
                                    xtic = xts[i // 4][
                                        :, (i % 4) * SEG + cc * 128:
                                        (i % 4) * SEG + (cc + 1) * 128]
                                    for half in range(2):
                                        nc.tensor.matmul(
                                            ps[c * 2 + half][:],
                                            xtic,
                                            w_sb[:, i * CH + half * 512:
                                                 i * CH + half * 512 + 512],
                                            start=(i == 0), stop=(i == 15))
                            for c in range(2):
                                cc = wave * 2 + c
                                for half in range(2):
                                    nc.scalar.copy(
                                        dests[cc][:, half * 512:
                                                  half * 512 + 512],
                                        ps[c * 2 + half][:])
                        return dests

                    qT = proj_T(wq_sb, "qT")
                    kT = proj_T(wk_sb, "kT")
                    v = proj_N(wv_sb, "v")

                    attT = att_pool.tile([128, HPP * SEG], f16, tag="attT")
                    attTs.append(attT)

                    for h in range(HPP):
                        memh, zf32 = mem_sb[(b, h)]

                        def elu1(src, dtag):
                            mn = tmp_pool.tile([128, SEG], f32, tag="tmp")
                            nc.vector.tensor_scalar_min(mn[:], src[:], 0.0)
                            e = tmp_pool.tile([128, SEG], f32, tag="tmp")
                            nc.scalar.activation(e[:], mn[:], ACTF.Exp)
                            r = tmp_pool.tile([128, SEG], f32, tag="tmp")
                            nc.scalar.activation(r[:], src[:], ACTF.Relu)
                            out = sig_pool.tile([128, SEG], f16, tag=dtag)
                            nc.vector.tensor_add(out[:], e[:], r[:])
                            return out

                        sgq = elu1(qT[h], "sgq")
                        sgk = elu1(kT[h], "sgk")
                        zsum = tiny_pool.tile([128, 1], f32, tag="zsum")
                        nc.vector.reduce_sum(zsum[:], sgk[:], axis=AXIS.X)
                        signat = sig_pool.tile([128, SEG], f16, tag="signat")
                        for c4 in range(4):
                            pt = pa.tile([128, 128], f16, tag="aux")
                            nc.tensor.transpose(
                                pt[:],
                                sgk[:, c4 * 128:(c4 + 1) * 128],
                                ident16[:])
                            nc.vector.tensor_copy(
                                signat[:, c4 * 128:(c4 + 1) * 128], pt[:])

                        es = []
                        for c4 in range(4):
                            psc = pp.tile([128, SEG], f32, tag="mm")
                            nc.tensor.matmul(psc[:],
                                             kT[h][:, c4 * 128:(c4 + 1) * 128],
                                             qT[h][:])
                            e = exps_pool.tile([128, SEG], f16, tag="exps")
                            nc.scalar.activation(e[:], psc[:], ACTF.Exp,
                                                 scale=SCALE)
                            nc.gpsimd.affine_select(
                                e[:], e[:], pattern=[[1, SEG]],
                                compare_op=ALU.is_ge, fill=0.0,
                                base=-c4 * 128, channel_multiplier=-1)
                            es.append(e)

                        pden = pa.tile([32, SEG], f32, tag="aux")
                        for c4 in range(4):
                            nc.tensor.matmul(pden[:], ones32[:], es[c4][:],
                                             start=(c4 == 0), stop=(c4 == 3))
                        pU = pp.tile([128, SEG], f32, tag="mm")
                        for c4 in range(4):
                            nc.tensor.matmul(pU[:],
                                             v[c4][:, h * 128:(h + 1) * 128],
                                             es[c4][:],
                                             start=(c4 == 0), stop=(c4 == 3))
                        pR = pp.tile([128, SEG], f32, tag="mm")
                        nc.tensor.matmul(pR[:], memh[:, 0:128], sgq[:])
                        zrep = tiny_pool.tile([128, 32], f16, tag="zrep")
                        nc.vector.tensor_scalar_mul(
                            zrep[:], ones32f[:], zf32[:, 0:1])
                        pzd = pa.tile([32, SEG], f32, tag="aux")
                        nc.tensor.matmul(pzd[:], zrep[:], sgq[:])

                        rden = rv_pool.tile([32, SEG], f32r, tag="rvec")
                        rzden = rv_pool.tile([32, SEG], f32r, tag="rvec")
                        with nc.allow_low_precision(
                                reason="fp32r for PE broadcast"):
                            nc.vector.reciprocal(rden[:], pden[:])
                            nc.vector.reciprocal(rzden[:], pzd[:])
                        pbd = pp.tile([128, SEG], f32, tag="mm")
                        nc.tensor.matmul(pbd[:], inv32[:], rden[:])
                        pbz = pp.tile([128, SEG], f32, tag="mm")
                        nc.tensor.matmul(pbz[:], inv32[:], rzden[:])

                        bd = tmp_pool.tile([128, SEG], f32, tag="tmp")
                        nc.scalar.copy(bd[:], pbd[:])
                        bz = tmp_pool.tile([128, SEG], f32, tag="tmp")
                        nc.scalar.copy(bz[:], pbz[:])
                        t1 = tmp_pool.tile([128, SEG], f32, tag="tmp")
                        nc.vector.tensor_tensor(t1[:], pU[:], bd[:],
                                                op=ALU.mult)
                        t2 = tmp_pool.tile([128, SEG], f32, tag="tmp")
                        nc.vector.tensor_tensor(t2[:], pR[:], bz[:],
                                                op=ALU.mult)
                        nc.vector.tensor_sub(t2[:], t2[:], t1[:])
                        nc.vector.scalar_tensor_tensor(
                            attT[:, h * SEG:(h + 1) * SEG],
                            t2[:], beta_sb[:, ph * HPP + h:ph * HPP + h + 1],
                            t1[:], op0=ALU.mult, op1=ALU.add)

                        pmu = pa.tile([128, 128], f32, tag="aux")
                        for c4 in range(4):
                            prz = pa.tile([128, 256], f32, tag="aux")
                            nc.tensor.matmul(prz[:],
                                             sgk[:, c4 * 128:(c4 + 1) * 128],
                                             memh[:])
                            rk = tiny_pool.tile([128, 1], f32, tag="rk")
                            nc.vector.reciprocal(rk[:], prz[:, 128:129])
                            nd = nd_pool.tile([128, 128], f16, tag="nd")
                            nc.vector.scalar_tensor_tensor(
                                nd[:], prz[:, 0:128], rk[:],
                                v[c4][:, h * 128:(h + 1) * 128],
                                op0=ALU.mult, op1=ALU.subtract)
                            nc.tensor.matmul(pmu[:],
                                             signat[:, c4 * 128:(c4 + 1) * 128],
                                             nd[:],
                                             start=(c4 == 0), stop=(c4 == 3))
                        nc.vector.tensor_sub(memh[:, 0:128], memh[:, 0:128],
                                             pmu[:])
                        nc.vector.tensor_tensor(memh[:, 128:129],
                                                memh[:, 128:129],
                                                zsum[:], op=ALU.add)
                        nc.vector.tensor_tensor(zf32[:], zf32[:],
                                                zsum[:], op=ALU.add)

                # ---- joint output projection for both batches: Wo is read
                # once per (phase, seg, col-half) instead of once per batch.
                # 8 PSUM accumulators = 2 batches x 2 rowblocks x 2 o-chunks.
                attv = [
                    attTs[b][:, rb * 4 * SEG:(rb + 1) * 4 * SEG]
                    .rearrange("p (h g j) -> p h g j", h=4, g=32, j=16)
                    for b in range(B) for rb in range(2)]
                for half in range(2):
                    po = [pp.tile([128, 512], f32, tag="mm",
                                  name=f"po{k}") for k in range(5)]
                    po += [pa.tile([128, 512], f32, tag="aux",
                                   name=f"po{5 + k}") for k in range(3)]
                    for j in range(16):
                        wot = wo_pool.tile([128, D // 2], f16, tag="wo")
                        nc.scalar.dma_start(
                            out=wot[:],
                            in_=wo_d[j * 128:(j + 1) * 128,
                                     half * 1024:(half + 1) * 1024])
                        for br in range(4):
                            for oh in range(2):
                                nc.tensor.matmul(
                                    po[br * 2 + oh][:],
                                    attv[br][:, :, :, j],
                                    wot[:, oh * 512:(oh + 1) * 512],
                                    start=(j == 0), stop=(j == 15))
                    for br in range(4):
                        b_, rb = divmod(br, 2)
                        osb = out_pool.tile([128, D // 2], f16, tag="outs")
                        nc.scalar.copy(osb[:, 0:512], po[br * 2][:])
                        nc.vector.tensor_copy(osb[:, 512:1024],
                                              po[br * 2 + 1][:])
                        r0 = 256 * ph + 128 * rb
                        nc.scalar.dma_start(
                            out=out_d[b_, seg, r0:r0 + 128,
                                      half * 1024:(half + 1) * 1024],
                            in_=osb[:])


def get_module():
    if "nc" not in _MODULE_CACHE:
        _MODULE_CACHE["nc"] = _build_module()
    return _MODULE_CACHE["nc"]


def _tile_w(W):
    """[D, D] -> [NPH, 128, 16*CH]: w[ph, p, i*CH+c] = W[i*128+p, ph*CH+c]."""
    w = np.asarray(W, np.float32).astype(np.float16)
    w = w.reshape(16, 128, NPH, CH).transpose(2, 1, 0, 3)
    return np.ascontiguousarray(w.reshape(NPH, 128, 16 * CH))


def make_in_maps(x, Wq, Wk, Wv, Wo, betas):
    x = np.asarray(x, np.float32).astype(np.float16)
    # xT[b, seg, p, i*SEG+t] = x[b, seg*SEG+t, i*128+p]  (16KB DMA lines)
    xT = x.reshape(B, NSEG, SEG, 16, 128).transpose(0, 1, 4, 3, 2)
    xT = np.ascontiguousarray(xT.reshape(B, NSEG, 128, 16 * SEG))
    beta_full = 1.0 / (1.0 + np.exp(-np.asarray(betas, np.float32)))
    return [{
        "xT": xT,
        "wq": _tile_w(Wq),
        "wk": _tile_w(Wk),
        "wv": _tile_w(Wv),
        "wo": np.ascontiguousarray(np.asarray(Wo, np.float32)
                                   .astype(np.float16)),
        "beta": np.ascontiguousarray(beta_full[0, :, 0, :].T),
    }]


def gather(results):
    out = results[0]["out"].astype(np.float32)  # [B, NSEG, 512, D]
    return out.reshape(B, S, D)


def kernel(x, Wq, Wk, Wv, Wo, betas):
    from concourse import bass2jax
    nc = get_module()
    in_maps = make_in_maps(x, Wq, Wk, Wv, Wo, betas)
    results = bass2jax.run_bass_via_pjrt(nc, in_maps, n_cores=NCORE)
    return gather(results)
